# revision 2
# baseline (speedup 1.0000x reference)
"""Trainium2 Bass kernel for nn_NodePreTrans (e3nn tensor product + linear).

Data-parallel over nodes: 50000 rows sharded 8 ways (6250/core, padded to
6272).  Channel-major layout; bf16 I/O + bf16 matmuls (f32 PSUM), merged
wide elementwise ops on DVE (scalar_tensor_tensor, 4x mode for SBUF bf16)
and merged PSUM evacuations on ACT.  PSUM: one 3-bank rotating pool (x2)
plus a 2-bank pool = 8 banks.
"""

import sys

sys.path.insert(0, "/opt/trn_rl_repo")

import numpy as np

import concourse.bacc as bacc
import concourse.bass as bass
import concourse.mybir as mybir
import concourse.tile as tile
from concourse.bass_utils import run_bass_kernel_spmd

N_NODES = 50000
N_CORES = 8
NS = N_NODES // N_CORES          # 6250 real nodes per core
NSH = 6272                       # padded (12*512 + 128)
TW = 512                         # PSUM bank width in f32

C_000 = 1.0 / np.sqrt(256.0)
C_011 = 1.0 / np.sqrt(128.0)
C_101 = 1.0 / np.sqrt(256.0)
C_110 = 1.0 / np.sqrt(384.0)
C_111 = 1.0 / 16.0

F32 = mybir.dt.float32
BF16 = mybir.dt.bfloat16
AOP = mybir.AluOpType

_CACHE = {}


def _build_program():
    nc = bacc.Bacc("TRN2", target_bir_lowering=False, debug=False,
                   num_devices=N_CORES)

    xT_d = nc.dram_tensor("xT", [640, NSH], BF16, kind="ExternalInput").ap()
    wt000_d = nc.dram_tensor("wt000", [256, 256], BF16, kind="ExternalInput").ap()
    wt011_d = nc.dram_tensor("wt011", [128, 256], BF16, kind="ExternalInput").ap()
    wt101_d = nc.dram_tensor("wt101", [256, 128], BF16, kind="ExternalInput").ap()
    wt110_d = nc.dram_tensor("wt110", [128, 128], BF16, kind="ExternalInput").ap()
    wt111_d = nc.dram_tensor("wt111", [128, 128], BF16, kind="ExternalInput").ap()
    l0e_d = nc.dram_tensor("l0e", [384, 256], BF16, kind="ExternalInput").ap()
    l1o_d = nc.dram_tensor("l1o", [384, 128], BF16, kind="ExternalInput").ap()
    l1e_d = nc.dram_tensor("l1e", [128, 128], BF16, kind="ExternalInput").ap()
    outT_d = nc.dram_tensor("outT", [1024, NSH], BF16, kind="ExternalOutput").ap()

    with tile.TileContext(nc) as tc:
        _emit(tc, nc, xT_d, wt000_d, wt011_d, wt101_d, wt110_d, wt111_d,
              l0e_d, l1o_d, l1e_d, outT_d)

    nc.compile()
    return nc


def _emit(tc, nc, xT_d, wt000_d, wt011_d, wt101_d, wt110_d, wt111_d,
          l0e_d, l1o_d, l1e_d, outT_d):
    zblocks = [(i * 512, 512) for i in range(12)] + [(6144, 128)]

    def vmul(eng, out, a, b):
        # out = a * b via TensorScalarPtr (4x DVE mode when all-SBUF bf16)
        eng.scalar_tensor_tensor(out, a, 1.0, b, AOP.mult, AOP.mult)

    def vsubr(eng, out, a, b):
        # out = b - a  (= (a * -1) + b)
        eng.scalar_tensor_tensor(out, a, -1.0, b, AOP.mult, AOP.add)

    def vadd(eng, out, a, b):
        eng.scalar_tensor_tensor(out, a, 1.0, b, AOP.mult, AOP.add)

    def as3(ap, Z):
        return ap.rearrange("p (j z) -> p j z", z=Z)

    def bc3(ap, Z):
        # [128, Z] slice -> broadcast [128, 3, Z] (stride 0 over j)
        return ap.rearrange("p (o z) -> p o z", o=1).broadcast_to((128, 3, Z))

    with (
        tc.tile_pool(name="wpool", bufs=1) as wpool,
        tc.tile_pool(name="xin", bufs=3) as xin,
        tc.tile_pool(name="mid", bufs=2) as mid,
        tc.tile_pool(name="oev", bufs=2) as oev,
        tc.tile_pool(name="psX", bufs=2, space="PSUM") as psX,
        tc.tile_pool(name="psY", bufs=1, space="PSUM") as psY,
    ):
        # ---- resident weights (bf16), loaded via ACT's DMA queue ---------
        def wtile(name, dram_ap, rows, cols):
            t = wpool.tile([128, cols], BF16, name=name)
            nc.scalar.dma_start(t[:, :], dram_ap[rows:rows + 128, :])
            return t

        w111 = wtile("w111", wt111_d, 0, 128)
        w011 = wtile("w011", wt011_d, 0, 256)
        w000 = [wtile(f"w000_{k}", wt000_d, 128 * k, 256) for k in range(2)]
        w101 = [wtile(f"w101_{k}", wt101_d, 128 * k, 128) for k in range(2)]
        w110 = wtile("w110", wt110_d, 0, 128)
        L1e = wtile("l1e", l1e_d, 0, 128)
        L1o = [wtile(f"l1o_{k}", l1o_d, 128 * k, 128) for k in range(3)]
        L0e = [wtile(f"l0e_{k}", l0e_d, 128 * k, 256) for k in range(3)]

        for bi, (z0, Z) in enumerate(zblocks):
            Z2, Z3 = 2 * Z, 3 * Z

            # ---- loads: v (3 groups) and s (2 groups), channel-major ----
            vcat = xin.tile([128, 3 * TW], BF16, name="vcat")
            nc.sync.dma_start(
                as3(vcat[:, :Z3], Z),
                xT_d[256:640, z0:z0 + Z].rearrange("(j p) z -> p j z", p=128))
            scat = xin.tile([128, 2 * TW], BF16, name="scat")
            nc.sync.dma_start(
                as3(scat[:, :Z2], Z),
                xT_d[0:256, z0:z0 + Z].rearrange("(m p) z -> p m z", p=128))

            def mm(out, w, rhs, start=True, stop=True):
                nc.tensor.matmul(out, w, rhs, start=start, stop=stop)

            def ps3(name):
                return psX.tile([128, 3 * TW], F32, name=name, tag="x3")

            # ---- path 5 stage 1: Ecat = [E2|E0|E1] ----------------------
            Ecat = ps3("Ecat")
            mm(Ecat[:, 0:Z], w111[:, :], vcat[:, Z2:Z3])
            mm(Ecat[:, Z:Z2], w111[:, :], vcat[:, 0:Z])
            mm(Ecat[:, Z2:Z3], w111[:, :], vcat[:, Z:Z2])
            Es = mid.tile([128, 3 * TW], BF16, name="Es")
            nc.scalar.copy(Es[:, :Z3], Ecat[:, :Z3])

            # ta_k = v_{(1,2,0)[k]} * E_{(2,0,1)[k]}; stored E order makes
            # E side contiguous; v side splits into 2 ranges (same for tb).
            ta = mid.tile([128, 3 * TW], BF16, name="ta")
            vmul(nc.vector, ta[:, 0:Z2], vcat[:, Z:Z3], Es[:, 0:Z2])
            vmul(nc.vector, ta[:, Z2:Z3], vcat[:, 0:Z], Es[:, Z2:Z3])
            tb = mid.tile([128, 3 * TW], BF16, name="tb")
            vmul(nc.vector, tb[:, 0:Z], vcat[:, Z2:Z3], Es[:, Z2:Z3])
            vmul(nc.vector, tb[:, Z:Z3], vcat[:, 0:Z2], Es[:, 0:Z2])
            p5 = mid.tile([128, 3 * TW], BF16, name="p5")
            vsubr(nc.vector, p5[:, :Z3], tb[:, :Z3], ta[:, :Z3])

            # ---- path 2 stage 1: bm = w011_m^T v_j, p2_m = s_m (.) bm ---
            p2m = []
            for m in range(2):
                bm = ps3(f"bm{m}")
                for j in range(3):
                    mm(bm[:, j * Z:(j + 1) * Z],
                       w011[:, 128 * m:128 * (m + 1)],
                       vcat[:, j * Z:(j + 1) * Z])
                p2 = mid.tile([128, 3 * TW], BF16, name=f"p2m{m}")
                nc.vector.scalar_tensor_tensor(
                    as3(p2[:, :Z3], Z), as3(bm[:, :Z3], Z), 1.0,
                    bc3(scat[:, m * Z:(m + 1) * Z], Z), AOP.mult, AOP.mult)
                p2m.append(p2)

            # ---- paths 1+3 stage 1: acM = [a0|a1|c] ---------------------
            acM = ps3("acM")
            for m in range(2):
                mm(acM[:, m * Z:(m + 1) * Z],
                   w000[0][:, 128 * m:128 * (m + 1)], scat[:, 0:Z],
                   start=True, stop=False)
                mm(acM[:, m * Z:(m + 1) * Z],
                   w000[1][:, 128 * m:128 * (m + 1)], scat[:, Z:Z2],
                   start=False, stop=True)
            mm(acM[:, Z2:Z3], w101[0][:, :], scat[:, 0:Z],
               start=True, stop=False)
            mm(acM[:, Z2:Z3], w101[1][:, :], scat[:, Z:Z2],
               start=False, stop=True)

            p1 = mid.tile([128, 2 * TW], BF16, name="p1")
            vmul(nc.vector, p1[:, :Z2], acM[:, 0:Z2], scat[:, 0:Z2])
            cs = mid.tile([128, TW], BF16, name="cs")
            nc.scalar.copy(cs[:, :Z], acM[:, Z2:Z3])
            p3 = mid.tile([128, 3 * TW], BF16, name="p3")
            nc.vector.scalar_tensor_tensor(
                as3(p3[:, :Z3], Z), as3(vcat[:, :Z3], Z), 1.0,
                bc3(cs[:, 0:Z], Z), AOP.mult, AOP.mult)

            # ---- path 4 stage 1: dcat, t4 = v (.) d, p4 = sum_j t4_j ----
            dcat = ps3("dcat")
            for j in range(3):
                mm(dcat[:, j * Z:(j + 1) * Z], w110[:, :],
                   vcat[:, j * Z:(j + 1) * Z])
            ds = mid.tile([128, 3 * TW], BF16, name="ds")
            nc.scalar.copy(ds[:, :Z3], dcat[:, :Z3])
            t4 = mid.tile([128, 3 * TW], BF16, name="t4")
            vmul(nc.vector, t4[:, :Z3], ds[:, :Z3], vcat[:, :Z3])
            p4 = mid.tile([128, TW], BF16, name="p4")
            vadd(nc.vector, p4[:, :Z], t4[:, 0:Z], t4[:, Z:Z2])
            vadd(nc.vector, p4[:, :Z], p4[:, :Z], t4[:, Z2:Z3])

            # ---- stage 2 ------------------------------------------------
            def store3(ev, row0, nrow):
                nc.sync.dma_start(
                    outT_d[row0:row0 + nrow, z0:z0 + Z]
                    .rearrange("(j p) z -> p j z", p=128),
                    as3(ev[:, :Z * (nrow // 128)], Z))

            # 1o output: accumulate 3 chunks into o1o
            o1o = ps3("o1o")
            tp1o = [p2m[0], p2m[1], p3]
            if Z == 512:
                for ci in range(3):
                    for j in range(3):
                        mm(o1o[:, j * Z:(j + 1) * Z], L1o[ci][:, :],
                           tp1o[ci][:, j * Z:(j + 1) * Z],
                           start=(ci == 0), stop=(ci == 2))
            else:
                # tail: j-groups share a PSUM bank; keep each j's
                # accumulation group contiguous so start=True of the next
                # group doesn't clear has_written mid-accumulation.
                for j in range(3):
                    for ci in range(3):
                        mm(o1o[:, j * Z:(j + 1) * Z], L1o[ci][:, :],
                           tp1o[ci][:, j * Z:(j + 1) * Z],
                           start=(ci == 0), stop=(ci == 2))
            ev1o = oev.tile([128, 3 * TW], BF16, name="ev1o")
            nc.scalar.copy(ev1o[:, :Z3], o1o[:, :Z3])
            store3(ev1o, 256, 384)

            # 1e output
            o1e = ps3("o1e")
            for j in range(3):
                mm(o1e[:, j * Z:(j + 1) * Z], L1e[:, :],
                   p5[:, j * Z:(j + 1) * Z])
            ev1e = oev.tile([128, 3 * TW], BF16, name="ev1e")
            nc.scalar.copy(ev1e[:, :Z3], o1e[:, :Z3])
            store3(ev1e, 640, 384)

            # 0e output
            o0e = psY.tile([128, 2 * TW], F32, name="o0e", tag="y2")
            for m in range(2):
                mm(o0e[:, m * Z:(m + 1) * Z],
                   L0e[0][:, 128 * m:128 * (m + 1)], p1[:, 0:Z],
                   start=True, stop=False)
                mm(o0e[:, m * Z:(m + 1) * Z],
                   L0e[1][:, 128 * m:128 * (m + 1)], p1[:, Z:Z2],
                   start=False, stop=False)
                mm(o0e[:, m * Z:(m + 1) * Z],
                   L0e[2][:, 128 * m:128 * (m + 1)], p4[:, :Z],
                   start=False, stop=True)
            ev0e = oev.tile([128, 2 * TW], BF16, name="ev0e")
            nc.scalar.copy(ev0e[:, :Z2], o0e[:, :Z2])
            store3(ev0e, 0, 256)


def _prep_inputs(node_feat, w_00_0, w_01_1, w_10_1, w_11_0, w_11_1,
                 W_0e, W_1o, W_1e):
    import ml_dtypes
    ndt = ml_dtypes.bfloat16
    weights = {
        "wt000": np.ascontiguousarray((C_000 * w_00_0).T).astype(ndt),
        "wt011": np.ascontiguousarray((C_011 * w_01_1).T).astype(ndt),
        "wt101": np.ascontiguousarray((C_101 * w_10_1).T).astype(ndt),
        "wt110": np.ascontiguousarray((C_110 * w_11_0).T).astype(ndt),
        "wt111": np.ascontiguousarray((C_111 * w_11_1).T).astype(ndt),
        "l0e": np.ascontiguousarray(W_0e / np.sqrt(384.0)).astype(ndt),
        "l1o": np.ascontiguousarray(W_1o / np.sqrt(384.0)).astype(ndt),
        "l1e": np.ascontiguousarray(W_1e / np.sqrt(128.0)).astype(ndt),
    }
    feat = np.asarray(node_feat, dtype=np.float32).reshape(N_CORES, NS, 640)
    in_maps = []
    for i in range(N_CORES):
        blk = feat[i]
        xT = np.zeros((640, NSH), ndt)
        xT[:256, :NS] = blk[:, :256].T.astype(ndt)
        vv = blk[:, 256:].reshape(NS, 128, 3)
        xT[256:, :NS] = vv.transpose(2, 1, 0).reshape(384, NS).astype(ndt)
        in_maps.append({"xT": xT, **weights})
    return in_maps


def _gather(results):
    out = np.empty((N_NODES, 1024), np.float32)
    for i in range(N_CORES):
        oT = np.asarray(results[i]["outT"]).astype(np.float32,
                                                   copy=False)[:, :NS]
        blk = out[i * NS:(i + 1) * NS]
        blk[:, :256] = oT[:256].T
        blk[:, 256:640] = oT[256:640].reshape(3, 128, NS).transpose(2, 1, 0) \
            .reshape(NS, 384)
        blk[:, 640:] = oT[640:].reshape(3, 128, NS).transpose(2, 1, 0) \
            .reshape(NS, 384)
    return out


def kernel(node_feat, w_00_0, w_01_1, w_10_1, w_11_0, w_11_1,
           W_0e, W_1o, W_1e, _trace=False):
    if "v1" not in _CACHE:
        _CACHE["v1"] = _build_program()
    nc = _CACHE["v1"]
    in_maps = _prep_inputs(node_feat, w_00_0, w_01_1, w_10_1, w_11_0,
                           w_11_1, W_0e, W_1o, W_1e)
    res = run_bass_kernel_spmd(nc, in_maps, core_ids=list(range(N_CORES)),
                               trace=_trace)
    out = _gather(res.results)
    if _trace:
        return out, res
    return out


# revision 7
# speedup vs baseline: 1.0624x; 1.0624x over previous
"""Trainium2 Bass kernel for nn_NodePreTrans (e3nn tensor product + linear).

Data-parallel over nodes: 50000 rows sharded 8 ways (6250/core, padded to
6272).  Channel-major layout; bf16 I/O + bf16 matmuls (f32 PSUM), merged
wide elementwise ops on DVE (scalar_tensor_tensor, 4x mode for SBUF bf16)
and merged PSUM evacuations on ACT.  PSUM: one 3-bank rotating pool (x2)
plus a 2-bank pool = 8 banks.
"""

import sys

sys.path.insert(0, "/opt/trn_rl_repo")

import numpy as np

import concourse.bacc as bacc
import concourse.bass as bass
import concourse.mybir as mybir
import concourse.tile as tile
from concourse.bass_utils import run_bass_kernel_spmd

N_NODES = 50000
N_CORES = 8
NS = N_NODES // N_CORES          # 6250 real nodes per core
NSH = 6272                       # padded (12*512 + 128)
TW = 512                         # PSUM bank width in f32

C_000 = 1.0 / np.sqrt(256.0)
C_011 = 1.0 / np.sqrt(128.0)
C_101 = 1.0 / np.sqrt(256.0)
C_110 = 1.0 / np.sqrt(384.0)
C_111 = 1.0 / 16.0

F32 = mybir.dt.float32
BF16 = mybir.dt.bfloat16
AOP = mybir.AluOpType

_CACHE = {}


def _build_program():
    nc = bacc.Bacc("TRN2", target_bir_lowering=False, debug=False,
                   num_devices=N_CORES)

    xT_d = nc.dram_tensor("xT", [640, NSH], BF16, kind="ExternalInput").ap()
    wt000_d = nc.dram_tensor("wt000", [256, 256], BF16, kind="ExternalInput").ap()
    wt011_d = nc.dram_tensor("wt011", [128, 256], BF16, kind="ExternalInput").ap()
    wt101_d = nc.dram_tensor("wt101", [256, 128], BF16, kind="ExternalInput").ap()
    wt110_d = nc.dram_tensor("wt110", [128, 128], BF16, kind="ExternalInput").ap()
    wt111_d = nc.dram_tensor("wt111", [128, 128], BF16, kind="ExternalInput").ap()
    l0e_d = nc.dram_tensor("l0e", [384, 256], BF16, kind="ExternalInput").ap()
    l1o_d = nc.dram_tensor("l1o", [384, 128], BF16, kind="ExternalInput").ap()
    l1e_d = nc.dram_tensor("l1e", [128, 128], BF16, kind="ExternalInput").ap()
    outT_d = nc.dram_tensor("outT", [1024, NSH], BF16, kind="ExternalOutput").ap()

    with tile.TileContext(nc) as tc:
        _emit(tc, nc, xT_d, wt000_d, wt011_d, wt101_d, wt110_d, wt111_d,
              l0e_d, l1o_d, l1e_d, outT_d)

    nc.compile()
    return nc


def _emit(tc, nc, xT_d, wt000_d, wt011_d, wt101_d, wt110_d, wt111_d,
          l0e_d, l1o_d, l1e_d, outT_d):
    zblocks = [(i * 512, 512) for i in range(12)] + [(6144, 128)]

    # plain tensor_tensor: the only 2-tensor op with a 2x_1p uop on HW
    def vmul(eng, out, a, b):
        eng.tensor_mul(out, a, b)

    def vsub(eng, out, a, b):
        eng.tensor_sub(out, a, b)

    def vadd(eng, out, a, b):
        eng.tensor_add(out, a, b)

    def as3(ap, Z):
        return ap.rearrange("p (j z) -> p j z", z=Z)

    def bc3(ap, Z):
        # [128, Z] slice -> broadcast [128, 3, Z] (stride 0 over j)
        return ap.rearrange("p (o z) -> p o z", o=1).broadcast_to((128, 3, Z))

    with (
        tc.tile_pool(name="wpool", bufs=1) as wpool,
        tc.tile_pool(name="xin", bufs=3) as xin,
        tc.tile_pool(name="mid", bufs=2) as mid,
        tc.tile_pool(name="oev", bufs=2) as oev,
        tc.tile_pool(name="psX", bufs=2, space="PSUM") as psX,
        tc.tile_pool(name="psY", bufs=1, space="PSUM") as psY,
    ):
        # ---- resident weights (bf16), loaded via ACT's DMA queue ---------
        def wtile(name, dram_ap, rows, cols):
            t = wpool.tile([128, cols], BF16, name=name)
            nc.scalar.dma_start(t[:, :], dram_ap[rows:rows + 128, :])
            return t

        w111 = wtile("w111", wt111_d, 0, 128)
        w011 = wtile("w011", wt011_d, 0, 256)
        w000 = [wtile(f"w000_{k}", wt000_d, 128 * k, 256) for k in range(2)]
        w101 = [wtile(f"w101_{k}", wt101_d, 128 * k, 128) for k in range(2)]
        w110 = wtile("w110", wt110_d, 0, 128)
        L1e = wtile("l1e", l1e_d, 0, 128)
        L1o = [wtile(f"l1o_{k}", l1o_d, 128 * k, 128) for k in range(3)]
        L0e = [wtile(f"l0e_{k}", l0e_d, 128 * k, 256) for k in range(3)]

        for bi, (z0, Z) in enumerate(zblocks):
            Z2, Z3 = 2 * Z, 3 * Z

            # ---- loads: v (3 groups) and s (2 groups), channel-major ----
            vcat = xin.tile([128, 3 * TW], BF16, name="vcat")
            nc.sync.dma_start(
                as3(vcat[:, :Z3], Z),
                xT_d[256:640, z0:z0 + Z].rearrange("(j p) z -> p j z", p=128))
            scat = xin.tile([128, 2 * TW], BF16, name="scat")
            nc.sync.dma_start(
                as3(scat[:, :Z2], Z),
                xT_d[0:256, z0:z0 + Z].rearrange("(m p) z -> p m z", p=128))

            def mm(out, w, rhs, start=True, stop=True):
                nc.tensor.matmul(out, w, rhs, start=start, stop=stop)

            def ps3(name):
                return psX.tile([128, 3 * TW], F32, name=name, tag="x3")

            # ---- path 5 stage 1: Ecat = [E2|E0|E1] ----------------------
            Ecat = ps3("Ecat")
            mm(Ecat[:, 0:Z], w111[:, :], vcat[:, Z2:Z3])
            mm(Ecat[:, Z:Z2], w111[:, :], vcat[:, 0:Z])
            mm(Ecat[:, Z2:Z3], w111[:, :], vcat[:, Z:Z2])
            Es = mid.tile([128, 3 * TW], BF16, name="Es")
            nc.scalar.copy(Es[:, :Z3], Ecat[:, :Z3])

            # ta_k = v_{(1,2,0)[k]} * E_{(2,0,1)[k]}; stored E order makes
            # E side contiguous; v side splits into 2 ranges (same for tb).
            ta = mid.tile([128, 3 * TW], BF16, name="ta")
            vmul(nc.vector, ta[:, 0:Z2], vcat[:, Z:Z3], Es[:, 0:Z2])
            vmul(nc.gpsimd, ta[:, Z2:Z3], vcat[:, 0:Z], Es[:, Z2:Z3])
            tb = mid.tile([128, 3 * TW], BF16, name="tb")
            vmul(nc.gpsimd, tb[:, 0:Z], vcat[:, Z2:Z3], Es[:, Z2:Z3])
            vmul(nc.vector, tb[:, Z:Z3], vcat[:, 0:Z2], Es[:, 0:Z2])
            p5 = mid.tile([128, 3 * TW], BF16, name="p5")
            vsub(nc.vector, p5[:, :Z3], ta[:, :Z3], tb[:, :Z3])

            # ---- path 2 stage 1: bm = w011_m^T v_j, p2_m = s_m (.) bm ---
            p2m = []
            for m in range(2):
                bm = ps3(f"bm{m}")
                for j in range(3):
                    mm(bm[:, j * Z:(j + 1) * Z],
                       w011[:, 128 * m:128 * (m + 1)],
                       vcat[:, j * Z:(j + 1) * Z])
                p2 = mid.tile([128, 3 * TW], BF16, name=f"p2m{m}")
                vmul(nc.vector, as3(p2[:, :Z3], Z), as3(bm[:, :Z3], Z),
                     bc3(scat[:, m * Z:(m + 1) * Z], Z))
                p2m.append(p2)

            # ---- paths 1+3 stage 1: acM = [a0|a1|c] ---------------------
            acM = ps3("acM")
            for m in range(2):
                mm(acM[:, m * Z:(m + 1) * Z],
                   w000[0][:, 128 * m:128 * (m + 1)], scat[:, 0:Z],
                   start=True, stop=False)
                mm(acM[:, m * Z:(m + 1) * Z],
                   w000[1][:, 128 * m:128 * (m + 1)], scat[:, Z:Z2],
                   start=False, stop=True)
            mm(acM[:, Z2:Z3], w101[0][:, :], scat[:, 0:Z],
               start=True, stop=False)
            mm(acM[:, Z2:Z3], w101[1][:, :], scat[:, Z:Z2],
               start=False, stop=True)

            p1 = mid.tile([128, 2 * TW], BF16, name="p1")
            vmul(nc.vector, p1[:, :Z2], acM[:, 0:Z2], scat[:, 0:Z2])
            cs = mid.tile([128, TW], BF16, name="cs")
            nc.scalar.copy(cs[:, :Z], acM[:, Z2:Z3])
            p3 = mid.tile([128, 3 * TW], BF16, name="p3")
            vmul(nc.vector, as3(p3[:, :Z3], Z), as3(vcat[:, :Z3], Z),
                 bc3(cs[:, 0:Z], Z))

            # ---- path 4 stage 1: dcat, t4 = v (.) d, p4 = sum_j t4_j ----
            dcat = ps3("dcat")
            for j in range(3):
                mm(dcat[:, j * Z:(j + 1) * Z], w110[:, :],
                   vcat[:, j * Z:(j + 1) * Z])
            ds = mid.tile([128, 3 * TW], BF16, name="ds")
            nc.scalar.copy(ds[:, :Z3], dcat[:, :Z3])
            t4 = mid.tile([128, 3 * TW], BF16, name="t4")
            vmul(nc.vector, t4[:, :Z3], ds[:, :Z3], vcat[:, :Z3])
            p4 = mid.tile([128, TW], BF16, name="p4")
            vadd(nc.gpsimd, p4[:, :Z], t4[:, 0:Z], t4[:, Z:Z2])
            vadd(nc.gpsimd, p4[:, :Z], p4[:, :Z], t4[:, Z2:Z3])

            # ---- stage 2 ------------------------------------------------
            def store3(ev, row0, nrow):
                nc.sync.dma_start(
                    outT_d[row0:row0 + nrow, z0:z0 + Z]
                    .rearrange("(j p) z -> p j z", p=128),
                    as3(ev[:, :Z * (nrow // 128)], Z))

            # 1o output: accumulate 3 chunks into o1o
            o1o = ps3("o1o")
            tp1o = [p2m[0], p2m[1], p3]
            if Z == 512:
                for ci in range(3):
                    for j in range(3):
                        mm(o1o[:, j * Z:(j + 1) * Z], L1o[ci][:, :],
                           tp1o[ci][:, j * Z:(j + 1) * Z],
                           start=(ci == 0), stop=(ci == 2))
            else:
                # tail: j-groups share a PSUM bank; keep each j's
                # accumulation group contiguous so start=True of the next
                # group doesn't clear has_written mid-accumulation.
                for j in range(3):
                    for ci in range(3):
                        mm(o1o[:, j * Z:(j + 1) * Z], L1o[ci][:, :],
                           tp1o[ci][:, j * Z:(j + 1) * Z],
                           start=(ci == 0), stop=(ci == 2))
            ev1o = oev.tile([128, 3 * TW], BF16, name="ev1o")
            nc.scalar.copy(ev1o[:, :Z3], o1o[:, :Z3])
            store3(ev1o, 256, 384)

            # 1e output
            o1e = ps3("o1e")
            for j in range(3):
                mm(o1e[:, j * Z:(j + 1) * Z], L1e[:, :],
                   p5[:, j * Z:(j + 1) * Z])
            ev1e = oev.tile([128, 3 * TW], BF16, name="ev1e")
            nc.scalar.copy(ev1e[:, :Z3], o1e[:, :Z3])
            store3(ev1e, 640, 384)

            # 0e output
            o0e = psY.tile([128, 2 * TW], F32, name="o0e", tag="y2")
            for m in range(2):
                mm(o0e[:, m * Z:(m + 1) * Z],
                   L0e[0][:, 128 * m:128 * (m + 1)], p1[:, 0:Z],
                   start=True, stop=False)
                mm(o0e[:, m * Z:(m + 1) * Z],
                   L0e[1][:, 128 * m:128 * (m + 1)], p1[:, Z:Z2],
                   start=False, stop=False)
                mm(o0e[:, m * Z:(m + 1) * Z],
                   L0e[2][:, 128 * m:128 * (m + 1)], p4[:, :Z],
                   start=False, stop=True)
            ev0e = oev.tile([128, 2 * TW], BF16, name="ev0e")
            nc.scalar.copy(ev0e[:, :Z2], o0e[:, :Z2])
            store3(ev0e, 0, 256)


def _prep_inputs(node_feat, w_00_0, w_01_1, w_10_1, w_11_0, w_11_1,
                 W_0e, W_1o, W_1e):
    import ml_dtypes
    ndt = ml_dtypes.bfloat16
    weights = {
        "wt000": np.ascontiguousarray((C_000 * w_00_0).T).astype(ndt),
        "wt011": np.ascontiguousarray((C_011 * w_01_1).T).astype(ndt),
        "wt101": np.ascontiguousarray((C_101 * w_10_1).T).astype(ndt),
        "wt110": np.ascontiguousarray((C_110 * w_11_0).T).astype(ndt),
        "wt111": np.ascontiguousarray((C_111 * w_11_1).T).astype(ndt),
        "l0e": np.ascontiguousarray(W_0e / np.sqrt(384.0)).astype(ndt),
        "l1o": np.ascontiguousarray(W_1o / np.sqrt(384.0)).astype(ndt),
        "l1e": np.ascontiguousarray(W_1e / np.sqrt(128.0)).astype(ndt),
    }
    feat = np.asarray(node_feat, dtype=np.float32).reshape(N_CORES, NS, 640)
    in_maps = []
    for i in range(N_CORES):
        blk = feat[i]
        xT = np.zeros((640, NSH), ndt)
        xT[:256, :NS] = blk[:, :256].T.astype(ndt)
        vv = blk[:, 256:].reshape(NS, 128, 3)
        xT[256:, :NS] = vv.transpose(2, 1, 0).reshape(384, NS).astype(ndt)
        in_maps.append({"xT": xT, **weights})
    return in_maps


def _gather(results):
    out = np.empty((N_NODES, 1024), np.float32)
    for i in range(N_CORES):
        oT = np.asarray(results[i]["outT"]).astype(np.float32,
                                                   copy=False)[:, :NS]
        blk = out[i * NS:(i + 1) * NS]
        blk[:, :256] = oT[:256].T
        blk[:, 256:640] = oT[256:640].reshape(3, 128, NS).transpose(2, 1, 0) \
            .reshape(NS, 384)
        blk[:, 640:] = oT[640:].reshape(3, 128, NS).transpose(2, 1, 0) \
            .reshape(NS, 384)
    return out


def kernel(node_feat, w_00_0, w_01_1, w_10_1, w_11_0, w_11_1,
           W_0e, W_1o, W_1e, _trace=False):
    if "v1" not in _CACHE:
        _CACHE["v1"] = _build_program()
    nc = _CACHE["v1"]
    in_maps = _prep_inputs(node_feat, w_00_0, w_01_1, w_10_1, w_11_0,
                           w_11_1, W_0e, W_1o, W_1e)
    res = run_bass_kernel_spmd(nc, in_maps, core_ids=list(range(N_CORES)),
                               trace=_trace)
    out = _gather(res.results)
    if _trace:
        return out, res
    return out


# revision 11
# speedup vs baseline: 1.1074x; 1.0423x over previous
"""Trainium2 Bass kernel for nn_NodePreTrans (e3nn tensor product + linear).

Data-parallel over nodes: 50000 rows sharded 8 ways (6250/core, padded to
6272).  Channel-major layout; bf16 I/O + bf16 matmuls (f32 PSUM), merged
wide elementwise ops on DVE (scalar_tensor_tensor, 4x mode for SBUF bf16)
and merged PSUM evacuations on ACT.  PSUM: one 3-bank rotating pool (x2)
plus a 2-bank pool = 8 banks.
"""

import sys

sys.path.insert(0, "/opt/trn_rl_repo")

import numpy as np

import concourse.bacc as bacc
import concourse.bass as bass
import concourse.mybir as mybir
import concourse.tile as tile
from concourse.bass_utils import run_bass_kernel_spmd

N_NODES = 50000
N_CORES = 8
NS = N_NODES // N_CORES          # 6250 real nodes per core
NSH = 6272                       # padded (12*512 + 128)
TW = 512                         # PSUM bank width in f32

C_000 = 1.0 / np.sqrt(256.0)
C_011 = 1.0 / np.sqrt(128.0)
C_101 = 1.0 / np.sqrt(256.0)
C_110 = 1.0 / np.sqrt(384.0)
C_111 = 1.0 / 16.0

F32 = mybir.dt.float32
BF16 = mybir.dt.bfloat16
AOP = mybir.AluOpType

_CACHE = {}


def _build_program():
    nc = bacc.Bacc("TRN2", target_bir_lowering=False, debug=False,
                   num_devices=N_CORES)

    xT_d = nc.dram_tensor("xT", [640, NSH], BF16, kind="ExternalInput").ap()
    wt000_d = nc.dram_tensor("wt000", [256, 256], BF16, kind="ExternalInput").ap()
    wt011_d = nc.dram_tensor("wt011", [128, 256], BF16, kind="ExternalInput").ap()
    wt101_d = nc.dram_tensor("wt101", [256, 128], BF16, kind="ExternalInput").ap()
    wt110_d = nc.dram_tensor("wt110", [128, 128], BF16, kind="ExternalInput").ap()
    wt111_d = nc.dram_tensor("wt111", [128, 128], BF16, kind="ExternalInput").ap()
    l0e_d = nc.dram_tensor("l0e", [384, 256], BF16, kind="ExternalInput").ap()
    l1o_d = nc.dram_tensor("l1o", [384, 128], BF16, kind="ExternalInput").ap()
    l1e_d = nc.dram_tensor("l1e", [128, 128], BF16, kind="ExternalInput").ap()
    outT_d = nc.dram_tensor("outT", [1024, NSH], BF16, kind="ExternalOutput").ap()

    with tile.TileContext(nc) as tc:
        _emit(tc, nc, xT_d, wt000_d, wt011_d, wt101_d, wt110_d, wt111_d,
              l0e_d, l1o_d, l1e_d, outT_d)

    nc.compile()
    return nc


def _emit(tc, nc, xT_d, wt000_d, wt011_d, wt101_d, wt110_d, wt111_d,
          l0e_d, l1o_d, l1e_d, outT_d):
    zblocks = [(i * 512, 512) for i in range(12)] + [(6144, 128)]

    # plain tensor_tensor: the only 2-tensor op with a 2x_1p uop on HW
    def vmul(eng, out, a, b):
        eng.tensor_mul(out, a, b)

    def vsub(eng, out, a, b):
        eng.tensor_sub(out, a, b)

    def vadd(eng, out, a, b):
        eng.tensor_add(out, a, b)

    def as3(ap, Z):
        return ap.rearrange("p (j z) -> p j z", z=Z)

    def bc3(ap, Z):
        # [128, Z] slice -> broadcast [128, 3, Z] (stride 0 over j)
        return ap.rearrange("p (o z) -> p o z", o=1).broadcast_to((128, 3, Z))

    with (
        tc.tile_pool(name="wpool", bufs=1) as wpool,
        tc.tile_pool(name="xin", bufs=3) as xin,
        tc.tile_pool(name="mid", bufs=3) as mid,
        tc.tile_pool(name="oev", bufs=2) as oev,
        tc.tile_pool(name="psX", bufs=2, space="PSUM") as psX,
        tc.tile_pool(name="psY", bufs=1, space="PSUM") as psY,
    ):
        # ---- resident weights (bf16), loaded via ACT's DMA queue ---------
        def wtile(name, dram_ap, rows, cols):
            t = wpool.tile([128, cols], BF16, name=name)
            nc.scalar.dma_start(t[:, :], dram_ap[rows:rows + 128, :])
            return t

        w111 = wtile("w111", wt111_d, 0, 128)
        w011 = wtile("w011", wt011_d, 0, 256)
        w000 = [wtile(f"w000_{k}", wt000_d, 128 * k, 256) for k in range(2)]
        w101 = [wtile(f"w101_{k}", wt101_d, 128 * k, 128) for k in range(2)]
        w110 = wtile("w110", wt110_d, 0, 128)
        L1e = wtile("l1e", l1e_d, 0, 128)
        L1o = [wtile(f"l1o_{k}", l1o_d, 128 * k, 128) for k in range(3)]
        L0e = [wtile(f"l0e_{k}", l0e_d, 128 * k, 256) for k in range(3)]

        def mm(out, w, rhs, start=True, stop=True):
            nc.tensor.matmul(out, w, rhs, start=start, stop=stop)

        def ps3(name):
            return psX.tile([128, 3 * TW], F32, name=name, tag="x3")

        def stage1(z0, Z):
            Z2, Z3 = 2 * Z, 3 * Z

            # ---- loads: v (3 groups) and s (2 groups), channel-major ----
            vcat = xin.tile([128, 3 * TW], BF16, name="vcat")
            nc.sync.dma_start(
                as3(vcat[:, :Z3], Z),
                xT_d[256:640, z0:z0 + Z].rearrange("(j p) z -> p j z", p=128))
            scat = xin.tile([128, 2 * TW], BF16, name="scat")
            nc.sync.dma_start(
                as3(scat[:, :Z2], Z),
                xT_d[0:256, z0:z0 + Z].rearrange("(m p) z -> p m z", p=128))

            # ---- path 5 stage 1: Ecat = [E2|E0|E1] ----------------------
            Ecat = ps3("Ecat")
            mm(Ecat[:, 0:Z], w111[:, :], vcat[:, Z2:Z3])
            mm(Ecat[:, Z:Z2], w111[:, :], vcat[:, 0:Z])
            mm(Ecat[:, Z2:Z3], w111[:, :], vcat[:, Z:Z2])
            Es = mid.tile([128, 3 * TW], BF16, name="Es")
            nc.scalar.copy(Es[:, :Z3], Ecat[:, :Z3])

            # ta_k = v_{(1,2,0)[k]} * E_{(2,0,1)[k]}; stored E order makes
            # E side contiguous; v side splits into 2 ranges (same for tb).
            ta = mid.tile([128, 3 * TW], BF16, name="ta")
            vmul(nc.vector, ta[:, 0:Z2], vcat[:, Z:Z3], Es[:, 0:Z2])
            vmul(nc.gpsimd, ta[:, Z2:Z3], vcat[:, 0:Z], Es[:, Z2:Z3])
            tb = mid.tile([128, 3 * TW], BF16, name="tb")
            vmul(nc.gpsimd, tb[:, 0:Z], vcat[:, Z2:Z3], Es[:, Z2:Z3])
            vmul(nc.vector, tb[:, Z:Z3], vcat[:, 0:Z2], Es[:, 0:Z2])
            p5 = mid.tile([128, 3 * TW], BF16, name="p5")
            vsub(nc.vector, p5[:, :Z3], ta[:, :Z3], tb[:, :Z3])

            # ---- path 2 stage 1: bm = w011_m^T v_j, p2_m = s_m (.) bm ---
            p2m = []
            for m in range(2):
                bm = ps3(f"bm{m}")
                for j in range(3):
                    mm(bm[:, j * Z:(j + 1) * Z],
                       w011[:, 128 * m:128 * (m + 1)],
                       vcat[:, j * Z:(j + 1) * Z])
                p2 = mid.tile([128, 3 * TW], BF16, name=f"p2m{m}")
                vmul(nc.vector, as3(p2[:, :Z3], Z), as3(bm[:, :Z3], Z),
                     bc3(scat[:, m * Z:(m + 1) * Z], Z))
                p2m.append(p2)

            # ---- paths 1+3 stage 1: acM = [a0|a1|c] ---------------------
            acM = ps3("acM")
            for m in range(2):
                mm(acM[:, m * Z:(m + 1) * Z],
                   w000[0][:, 128 * m:128 * (m + 1)], scat[:, 0:Z],
                   start=True, stop=False)
                mm(acM[:, m * Z:(m + 1) * Z],
                   w000[1][:, 128 * m:128 * (m + 1)], scat[:, Z:Z2],
                   start=False, stop=True)
            mm(acM[:, Z2:Z3], w101[0][:, :], scat[:, 0:Z],
               start=True, stop=False)
            mm(acM[:, Z2:Z3], w101[1][:, :], scat[:, Z:Z2],
               start=False, stop=True)

            p1 = mid.tile([128, 2 * TW], BF16, name="p1")
            vmul(nc.vector, p1[:, :Z2], acM[:, 0:Z2], scat[:, 0:Z2])
            cs = mid.tile([128, TW], BF16, name="cs")
            nc.scalar.copy(cs[:, :Z], acM[:, Z2:Z3])
            p3 = mid.tile([128, 3 * TW], BF16, name="p3")
            vmul(nc.vector, as3(p3[:, :Z3], Z), as3(vcat[:, :Z3], Z),
                 bc3(cs[:, 0:Z], Z))

            # ---- path 4 stage 1: dcat, t4 = v (.) d, p4 = sum_j t4_j ----
            dcat = ps3("dcat")
            for j in range(3):
                mm(dcat[:, j * Z:(j + 1) * Z], w110[:, :],
                   vcat[:, j * Z:(j + 1) * Z])
            ds = mid.tile([128, 3 * TW], BF16, name="ds")
            nc.scalar.copy(ds[:, :Z3], dcat[:, :Z3])
            t4 = mid.tile([128, 3 * TW], BF16, name="t4")
            vmul(nc.vector, t4[:, :Z3], ds[:, :Z3], vcat[:, :Z3])
            p4 = mid.tile([128, TW], BF16, name="p4")
            vadd(nc.gpsimd, p4[:, :Z], t4[:, 0:Z], t4[:, Z:Z2])
            vadd(nc.gpsimd, p4[:, :Z], p4[:, :Z], t4[:, Z2:Z3])

            return (z0, Z, p2m, p3, p5, p1, p4)

        def stage2(state):
            z0, Z, p2m, p3, p5, p1, p4 = state
            Z2, Z3 = 2 * Z, 3 * Z

            def store3(ev, row0, nrow):
                nc.sync.dma_start(
                    outT_d[row0:row0 + nrow, z0:z0 + Z]
                    .rearrange("(j p) z -> p j z", p=128),
                    as3(ev[:, :Z * (nrow // 128)], Z))

            # 1o output: accumulate 3 chunks into o1o
            o1o = ps3("o1o")
            tp1o = [p2m[0], p2m[1], p3]
            if Z == 512:
                for ci in range(3):
                    for j in range(3):
                        mm(o1o[:, j * Z:(j + 1) * Z], L1o[ci][:, :],
                           tp1o[ci][:, j * Z:(j + 1) * Z],
                           start=(ci == 0), stop=(ci == 2))
            else:
                # tail: j-groups share a PSUM bank; keep each j's
                # accumulation group contiguous so start=True of the next
                # group doesn't clear has_written mid-accumulation.
                for j in range(3):
                    for ci in range(3):
                        mm(o1o[:, j * Z:(j + 1) * Z], L1o[ci][:, :],
                           tp1o[ci][:, j * Z:(j + 1) * Z],
                           start=(ci == 0), stop=(ci == 2))
            ev1o = oev.tile([128, 3 * TW], BF16, name="ev1o")
            nc.scalar.copy(ev1o[:, :Z3], o1o[:, :Z3])
            store3(ev1o, 256, 384)

            # 1e output
            o1e = ps3("o1e")
            for j in range(3):
                mm(o1e[:, j * Z:(j + 1) * Z], L1e[:, :],
                   p5[:, j * Z:(j + 1) * Z])
            ev1e = oev.tile([128, 3 * TW], BF16, name="ev1e")
            nc.scalar.copy(ev1e[:, :Z3], o1e[:, :Z3])
            store3(ev1e, 640, 384)

            # 0e output
            o0e = psY.tile([128, 2 * TW], F32, name="o0e", tag="y2")
            for m in range(2):
                mm(o0e[:, m * Z:(m + 1) * Z],
                   L0e[0][:, 128 * m:128 * (m + 1)], p1[:, 0:Z],
                   start=True, stop=False)
                mm(o0e[:, m * Z:(m + 1) * Z],
                   L0e[1][:, 128 * m:128 * (m + 1)], p1[:, Z:Z2],
                   start=False, stop=False)
                mm(o0e[:, m * Z:(m + 1) * Z],
                   L0e[2][:, 128 * m:128 * (m + 1)], p4[:, :Z],
                   start=False, stop=True)
            ev0e = oev.tile([128, 2 * TW], BF16, name="ev0e")
            nc.scalar.copy(ev0e[:, :Z2], o0e[:, :Z2])
            store3(ev0e, 0, 256)

        # software pipeline: stage-1 of block i runs before stage-2 of
        # block i-1, giving every cross-engine dependency a block of slack
        prev = None
        for (z0, Z) in zblocks:
            st = stage1(z0, Z)
            if prev is not None:
                stage2(prev)
            prev = st
        stage2(prev)


def _prep_inputs(node_feat, w_00_0, w_01_1, w_10_1, w_11_0, w_11_1,
                 W_0e, W_1o, W_1e):
    import ml_dtypes
    ndt = ml_dtypes.bfloat16
    weights = {
        "wt000": np.ascontiguousarray((C_000 * w_00_0).T).astype(ndt),
        "wt011": np.ascontiguousarray((C_011 * w_01_1).T).astype(ndt),
        "wt101": np.ascontiguousarray((C_101 * w_10_1).T).astype(ndt),
        "wt110": np.ascontiguousarray((C_110 * w_11_0).T).astype(ndt),
        "wt111": np.ascontiguousarray((C_111 * w_11_1).T).astype(ndt),
        "l0e": np.ascontiguousarray(W_0e / np.sqrt(384.0)).astype(ndt),
        "l1o": np.ascontiguousarray(W_1o / np.sqrt(384.0)).astype(ndt),
        "l1e": np.ascontiguousarray(W_1e / np.sqrt(128.0)).astype(ndt),
    }
    feat = np.asarray(node_feat, dtype=np.float32).reshape(N_CORES, NS, 640)
    in_maps = []
    for i in range(N_CORES):
        blk = feat[i]
        xT = np.zeros((640, NSH), ndt)
        xT[:256, :NS] = blk[:, :256].T.astype(ndt)
        vv = blk[:, 256:].reshape(NS, 128, 3)
        xT[256:, :NS] = vv.transpose(2, 1, 0).reshape(384, NS).astype(ndt)
        in_maps.append({"xT": xT, **weights})
    return in_maps


def _gather(results):
    out = np.empty((N_NODES, 1024), np.float32)
    for i in range(N_CORES):
        oT = np.asarray(results[i]["outT"]).astype(np.float32,
                                                   copy=False)[:, :NS]
        blk = out[i * NS:(i + 1) * NS]
        blk[:, :256] = oT[:256].T
        blk[:, 256:640] = oT[256:640].reshape(3, 128, NS).transpose(2, 1, 0) \
            .reshape(NS, 384)
        blk[:, 640:] = oT[640:].reshape(3, 128, NS).transpose(2, 1, 0) \
            .reshape(NS, 384)
    return out


def kernel(node_feat, w_00_0, w_01_1, w_10_1, w_11_0, w_11_1,
           W_0e, W_1o, W_1e, _trace=False):
    if "v1" not in _CACHE:
        _CACHE["v1"] = _build_program()
    nc = _CACHE["v1"]
    in_maps = _prep_inputs(node_feat, w_00_0, w_01_1, w_10_1, w_11_0,
                           w_11_1, W_0e, W_1o, W_1e)
    res = run_bass_kernel_spmd(nc, in_maps, core_ids=list(range(N_CORES)),
                               trace=_trace)
    out = _gather(res.results)
    if _trace:
        return out, res
    return out


# revision 12
# speedup vs baseline: 1.4167x; 1.2793x over previous
"""Trainium2 Bass kernel for nn_NodePreTrans (e3nn tensor product + linear).

Data-parallel over nodes: 50000 rows sharded 8 ways (6250/core, padded to
6272).  Channel-major layout; bf16 I/O + bf16 matmuls (f32 PSUM), merged
wide elementwise ops on DVE (scalar_tensor_tensor, 4x mode for SBUF bf16)
and merged PSUM evacuations on ACT.  PSUM: one 3-bank rotating pool (x2)
plus a 2-bank pool = 8 banks.
"""

import sys

sys.path.insert(0, "/opt/trn_rl_repo")

import numpy as np

import concourse.bacc as bacc
import concourse.bass as bass
import concourse.mybir as mybir
import concourse.tile as tile
from concourse.bass_utils import run_bass_kernel_spmd

N_NODES = 50000
N_CORES = 8
NS = N_NODES // N_CORES          # 6250 real nodes per core
NSH = 6272                       # padded (12*512 + 128)
TW = 512                         # PSUM bank width in f32

C_000 = 1.0 / np.sqrt(256.0)
C_011 = 1.0 / np.sqrt(128.0)
C_101 = 1.0 / np.sqrt(256.0)
C_110 = 1.0 / np.sqrt(384.0)
C_111 = 1.0 / 16.0

F32 = mybir.dt.float32
BF16 = mybir.dt.bfloat16
AOP = mybir.AluOpType

_CACHE = {}


def _build_program():
    nc = bacc.Bacc("TRN2", target_bir_lowering=False, debug=False,
                   num_devices=N_CORES)

    xT_d = nc.dram_tensor("xT", [640, NSH], BF16, kind="ExternalInput").ap()
    wt000_d = nc.dram_tensor("wt000", [256, 256], BF16, kind="ExternalInput").ap()
    wt011_d = nc.dram_tensor("wt011", [128, 256], BF16, kind="ExternalInput").ap()
    wt101_d = nc.dram_tensor("wt101", [256, 128], BF16, kind="ExternalInput").ap()
    wt110_d = nc.dram_tensor("wt110", [128, 128], BF16, kind="ExternalInput").ap()
    wt111_d = nc.dram_tensor("wt111", [128, 128], BF16, kind="ExternalInput").ap()
    l0e_d = nc.dram_tensor("l0e", [384, 256], BF16, kind="ExternalInput").ap()
    l1o_d = nc.dram_tensor("l1o", [384, 128], BF16, kind="ExternalInput").ap()
    l1e_d = nc.dram_tensor("l1e", [128, 128], BF16, kind="ExternalInput").ap()
    outT_d = nc.dram_tensor("outT", [1024, NSH], BF16, kind="ExternalOutput").ap()

    with tile.TileContext(nc) as tc:
        _emit(tc, nc, xT_d, wt000_d, wt011_d, wt101_d, wt110_d, wt111_d,
              l0e_d, l1o_d, l1e_d, outT_d)

    nc.compile()
    return nc


def _emit(tc, nc, xT_d, wt000_d, wt011_d, wt101_d, wt110_d, wt111_d,
          l0e_d, l1o_d, l1e_d, outT_d):
    zblocks = [(i * 512, 512) for i in range(12)] + [(6144, 128)]

    # plain tensor_tensor: the only 2-tensor op with a 2x_1p uop on HW
    def vmul(eng, out, a, b):
        eng.tensor_mul(out, a, b)

    def as3(ap, Z):
        return ap.rearrange("p (j z) -> p j z", z=Z)

    def bc3(ap, Z):
        # [128, Z] slice -> broadcast [128, 3, Z] (stride 0 over j)
        return ap.rearrange("p (o z) -> p o z", o=1).broadcast_to((128, 3, Z))

    with (
        tc.tile_pool(name="wpool", bufs=1) as wpool,
        tc.tile_pool(name="xin", bufs=3) as xin,
        tc.tile_pool(name="mid", bufs=3) as mid,
        tc.tile_pool(name="oev", bufs=3) as oev,
        tc.tile_pool(name="psX", bufs=2, space="PSUM") as psX,
        tc.tile_pool(name="psY", bufs=2, space="PSUM") as psY,
    ):
        # ---- resident weights (bf16), loaded via ACT's DMA queue ---------
        def wtile(name, dram_ap, rows, cols):
            t = wpool.tile([128, cols], BF16, name=name)
            nc.scalar.dma_start(t[:, :], dram_ap[rows:rows + 128, :])
            return t

        w111 = wtile("w111", wt111_d, 0, 128)
        w011 = wtile("w011", wt011_d, 0, 256)
        w000 = [wtile(f"w000_{k}", wt000_d, 128 * k, 256) for k in range(2)]
        w101 = [wtile(f"w101_{k}", wt101_d, 128 * k, 128) for k in range(2)]
        w110 = wtile("w110", wt110_d, 0, 128)
        L1e = wtile("l1e", l1e_d, 0, 128)
        L1o = [wtile(f"l1o_{k}", l1o_d, 128 * k, 128) for k in range(3)]
        L0e = [wtile(f"l0e_{k}", l0e_d, 128 * k, 256) for k in range(3)]

        def mm(out, w, rhs, start=True, stop=True):
            nc.tensor.matmul(out, w, rhs, start=start, stop=stop)

        def ps3(name):
            return psX.tile([128, 3 * TW], F32, name=name, tag="x3")

        def ps1(name):
            return psY.tile([128, TW], F32, name=name, tag="y1")

        # ---------------- per-block pieces -------------------------------
        def loads(z0, Z):
            Z2, Z3 = 2 * Z, 3 * Z
            vcat = xin.tile([128, 3 * TW], BF16, name="vcat")
            nc.sync.dma_start(
                as3(vcat[:, :Z3], Z),
                xT_d[256:640, z0:z0 + Z].rearrange("(j p) z -> p j z", p=128))
            scat = xin.tile([128, 2 * TW], BF16, name="scat")
            nc.sync.dma_start(
                as3(scat[:, :Z2], Z),
                xT_d[0:256, z0:z0 + Z].rearrange("(m p) z -> p m z", p=128))
            return vcat, scat

        # stage-2 output groups: 1-bank PSUM tiles, evac on ACT, per-group
        # store on Sync.  Each is a generator-style callable so groups can
        # be interleaved into stage-1's natural PE stall points.
        def out_1o(st, j):
            z0, Z, vcat, scat, p2m, p3, p5, p1, p4 = st
            o = ps1(f"o1o_{j}")
            tp1o = [p2m[0], p2m[1], p3]
            for ci in range(3):
                mm(o[:, :Z], L1o[ci][:, :], tp1o[ci][:, j * Z:(j + 1) * Z],
                   start=(ci == 0), stop=(ci == 2))
            ev = oev.tile([128, TW], BF16, name=f"e1o_{j}", tag="ev")
            nc.scalar.copy(ev[:, :Z], o[:, :Z])
            nc.sync.dma_start(outT_d[256 + 128 * j:384 + 128 * j,
                                     z0:z0 + Z], ev[:, :Z])

        def out_1e(st, j):
            z0, Z, vcat, scat, p2m, p3, p5, p1, p4 = st
            o = ps1(f"o1e_{j}")
            mm(o[:, :Z], L1e[:, :], p5[:, j * Z:(j + 1) * Z])
            ev = oev.tile([128, TW], BF16, name=f"e1e_{j}", tag="ev")
            nc.scalar.copy(ev[:, :Z], o[:, :Z])
            nc.sync.dma_start(outT_d[640 + 128 * j:768 + 128 * j,
                                     z0:z0 + Z], ev[:, :Z])

        def out_0e(st, m):
            z0, Z, vcat, scat, p2m, p3, p5, p1, p4 = st
            o = ps1(f"o0e_{m}")
            mm(o[:, :Z], L0e[0][:, 128 * m:128 * (m + 1)], p1[:, 0:Z],
               start=True, stop=False)
            mm(o[:, :Z], L0e[1][:, 128 * m:128 * (m + 1)], p1[:, Z:2 * Z],
               start=False, stop=False)
            mm(o[:, :Z], L0e[2][:, 128 * m:128 * (m + 1)], p4[:, :Z],
               start=False, stop=True)
            ev = oev.tile([128, TW], BF16, name=f"e0e_{m}", tag="ev")
            nc.scalar.copy(ev[:, :Z], o[:, :Z])
            nc.sync.dma_start(outT_d[128 * m:128 * (m + 1), z0:z0 + Z],
                              ev[:, :Z])

        def iteration(z0, Z, prev):
            """Emit stage-1 of this block, with stage-2 of the previous
            block's groups interleaved into PE wait windows."""
            Z2, Z3 = 2 * Z, 3 * Z
            vcat, scat = loads(z0, Z)

            # E matmuls; stored [E2|E0|E1] so the p5 cross products are
            # contiguous-range elementwise ops
            Ecat = ps3("Ecat")
            mm(Ecat[:, 0:Z], w111[:, :], vcat[:, Z2:Z3])
            mm(Ecat[:, Z:Z2], w111[:, :], vcat[:, 0:Z])
            mm(Ecat[:, Z2:Z3], w111[:, :], vcat[:, Z:Z2])
            Es = mid.tile([128, 3 * TW], BF16, name="Es")
            nc.scalar.copy(Es[:, :Z3], Ecat[:, :Z3])

            # b matmuls for m=0 (p2 mul emitted right after, to free bank)
            bm0 = ps3("bm0")
            for j in range(3):
                mm(bm0[:, j * Z:(j + 1) * Z], w011[:, 0:128],
                   vcat[:, j * Z:(j + 1) * Z])
            p2m = [mid.tile([128, 3 * TW], BF16, name="p2m0")]
            vmul(nc.vector, as3(p2m[0][:, :Z3], Z), as3(bm0[:, :Z3], Z),
                 bc3(scat[:, 0:Z], Z))

            # fill PE while Es/p2m0 drain: previous block's 1o outputs
            if prev is not None:
                for j in range(3):
                    out_1o(prev, j)

            bm1 = ps3("bm1")
            for j in range(3):
                mm(bm1[:, j * Z:(j + 1) * Z], w011[:, 128:256],
                   vcat[:, j * Z:(j + 1) * Z])
            p2m.append(mid.tile([128, 3 * TW], BF16, name="p2m1"))
            vmul(nc.vector, as3(p2m[1][:, :Z3], Z), as3(bm1[:, :Z3], Z),
                 bc3(scat[:, Z:Z2], Z))

            # acM = [a0|a1|c]
            acM = ps3("acM")
            for m in range(2):
                mm(acM[:, m * Z:(m + 1) * Z],
                   w000[0][:, 128 * m:128 * (m + 1)], scat[:, 0:Z],
                   start=True, stop=False)
                mm(acM[:, m * Z:(m + 1) * Z],
                   w000[1][:, 128 * m:128 * (m + 1)], scat[:, Z:Z2],
                   start=False, stop=True)
            mm(acM[:, Z2:Z3], w101[0][:, :], scat[:, 0:Z],
               start=True, stop=False)
            mm(acM[:, Z2:Z3], w101[1][:, :], scat[:, Z:Z2],
               start=False, stop=True)
            p1 = mid.tile([128, 2 * TW], BF16, name="p1")
            vmul(nc.vector, p1[:, :Z2], acM[:, 0:Z2], scat[:, 0:Z2])
            cs = mid.tile([128, TW], BF16, name="cs")
            nc.scalar.copy(cs[:, :Z], acM[:, Z2:Z3])

            # previous block's 1e outputs
            if prev is not None:
                for j in range(3):
                    out_1e(prev, j)

            # d matmuls
            dcat = ps3("dcat")
            for j in range(3):
                mm(dcat[:, j * Z:(j + 1) * Z], w110[:, :],
                   vcat[:, j * Z:(j + 1) * Z])
            ds = mid.tile([128, 3 * TW], BF16, name="ds")
            nc.scalar.copy(ds[:, :Z3], dcat[:, :Z3])

            # previous block's 0e outputs
            if prev is not None:
                for m in range(2):
                    out_0e(prev, m)

            # SBUF-side elementwise (DVE 2x_1p / GpSimd)
            ta = mid.tile([128, 3 * TW], BF16, name="ta")
            vmul(nc.vector, ta[:, 0:Z2], vcat[:, Z:Z3], Es[:, 0:Z2])
            vmul(nc.gpsimd, ta[:, Z2:Z3], vcat[:, 0:Z], Es[:, Z2:Z3])
            tb = mid.tile([128, 3 * TW], BF16, name="tb")
            vmul(nc.gpsimd, tb[:, 0:Z], vcat[:, Z2:Z3], Es[:, Z2:Z3])
            vmul(nc.vector, tb[:, Z:Z3], vcat[:, 0:Z2], Es[:, 0:Z2])
            p5 = mid.tile([128, 3 * TW], BF16, name="p5")
            nc.vector.tensor_sub(p5[:, :Z3], ta[:, :Z3], tb[:, :Z3])

            p3 = mid.tile([128, 3 * TW], BF16, name="p3")
            vmul(nc.vector, as3(p3[:, :Z3], Z), as3(vcat[:, :Z3], Z),
                 bc3(cs[:, 0:Z], Z))
            t4 = mid.tile([128, 3 * TW], BF16, name="t4")
            vmul(nc.vector, t4[:, :Z3], ds[:, :Z3], vcat[:, :Z3])
            p4 = mid.tile([128, TW], BF16, name="p4")
            nc.gpsimd.tensor_add(p4[:, :Z], t4[:, 0:Z], t4[:, Z:Z2])
            nc.gpsimd.tensor_add(p4[:, :Z], p4[:, :Z], t4[:, Z2:Z3])

            return (z0, Z, vcat, scat, p2m, p3, p5, p1, p4)

        prev = None
        for (z0, Z) in zblocks:
            prev = iteration(z0, Z, prev)
        for j in range(3):
            out_1o(prev, j)
        for j in range(3):
            out_1e(prev, j)
        for m in range(2):
            out_0e(prev, m)


def _prep_inputs(node_feat, w_00_0, w_01_1, w_10_1, w_11_0, w_11_1,
                 W_0e, W_1o, W_1e):
    import ml_dtypes
    ndt = ml_dtypes.bfloat16
    weights = {
        "wt000": np.ascontiguousarray((C_000 * w_00_0).T).astype(ndt),
        "wt011": np.ascontiguousarray((C_011 * w_01_1).T).astype(ndt),
        "wt101": np.ascontiguousarray((C_101 * w_10_1).T).astype(ndt),
        "wt110": np.ascontiguousarray((C_110 * w_11_0).T).astype(ndt),
        "wt111": np.ascontiguousarray((C_111 * w_11_1).T).astype(ndt),
        "l0e": np.ascontiguousarray(W_0e / np.sqrt(384.0)).astype(ndt),
        "l1o": np.ascontiguousarray(W_1o / np.sqrt(384.0)).astype(ndt),
        "l1e": np.ascontiguousarray(W_1e / np.sqrt(128.0)).astype(ndt),
    }
    feat = np.asarray(node_feat, dtype=np.float32).reshape(N_CORES, NS, 640)
    in_maps = []
    for i in range(N_CORES):
        blk = feat[i]
        xT = np.zeros((640, NSH), ndt)
        xT[:256, :NS] = blk[:, :256].T.astype(ndt)
        vv = blk[:, 256:].reshape(NS, 128, 3)
        xT[256:, :NS] = vv.transpose(2, 1, 0).reshape(384, NS).astype(ndt)
        in_maps.append({"xT": xT, **weights})
    return in_maps


def _gather(results):
    out = np.empty((N_NODES, 1024), np.float32)
    for i in range(N_CORES):
        oT = np.asarray(results[i]["outT"]).astype(np.float32,
                                                   copy=False)[:, :NS]
        blk = out[i * NS:(i + 1) * NS]
        blk[:, :256] = oT[:256].T
        blk[:, 256:640] = oT[256:640].reshape(3, 128, NS).transpose(2, 1, 0) \
            .reshape(NS, 384)
        blk[:, 640:] = oT[640:].reshape(3, 128, NS).transpose(2, 1, 0) \
            .reshape(NS, 384)
    return out


def kernel(node_feat, w_00_0, w_01_1, w_10_1, w_11_0, w_11_1,
           W_0e, W_1o, W_1e, _trace=False):
    if "v1" not in _CACHE:
        _CACHE["v1"] = _build_program()
    nc = _CACHE["v1"]
    in_maps = _prep_inputs(node_feat, w_00_0, w_01_1, w_10_1, w_11_0,
                           w_11_1, W_0e, W_1o, W_1e)
    res = run_bass_kernel_spmd(nc, in_maps, core_ids=list(range(N_CORES)),
                               trace=_trace)
    out = _gather(res.results)
    if _trace:
        return out, res
    return out


# revision 13
# speedup vs baseline: 1.5415x; 1.0881x over previous
"""Trainium2 Bass kernel for nn_NodePreTrans (e3nn tensor product + linear).

Data-parallel over nodes: 50000 rows sharded 8 ways (6250/core, padded to
6272).  Channel-major layout; bf16 I/O + bf16 matmuls (f32 PSUM), merged
wide elementwise ops on DVE (scalar_tensor_tensor, 4x mode for SBUF bf16)
and merged PSUM evacuations on ACT.  PSUM: one 3-bank rotating pool (x2)
plus a 2-bank pool = 8 banks.
"""

import sys

sys.path.insert(0, "/opt/trn_rl_repo")

import numpy as np

import concourse.bacc as bacc
import concourse.bass as bass
import concourse.mybir as mybir
import concourse.tile as tile
from concourse.bass_utils import run_bass_kernel_spmd

N_NODES = 50000
N_CORES = 8
NS = N_NODES // N_CORES          # 6250 real nodes per core
NSH = 6272                       # padded (12*512 + 128)
TW = 512                         # PSUM bank width in f32

C_000 = 1.0 / np.sqrt(256.0)
C_011 = 1.0 / np.sqrt(128.0)
C_101 = 1.0 / np.sqrt(256.0)
C_110 = 1.0 / np.sqrt(384.0)
C_111 = 1.0 / 16.0

F32 = mybir.dt.float32
BF16 = mybir.dt.bfloat16
AOP = mybir.AluOpType

_CACHE = {}


def _build_program():
    nc = bacc.Bacc("TRN2", target_bir_lowering=False, debug=False,
                   num_devices=N_CORES)

    xT_d = nc.dram_tensor("xT", [640, NSH], BF16, kind="ExternalInput").ap()
    wt000_d = nc.dram_tensor("wt000", [256, 256], BF16, kind="ExternalInput").ap()
    wt011_d = nc.dram_tensor("wt011", [128, 256], BF16, kind="ExternalInput").ap()
    wt101_d = nc.dram_tensor("wt101", [256, 128], BF16, kind="ExternalInput").ap()
    wt110_d = nc.dram_tensor("wt110", [128, 128], BF16, kind="ExternalInput").ap()
    wt111_d = nc.dram_tensor("wt111", [128, 128], BF16, kind="ExternalInput").ap()
    l0e_d = nc.dram_tensor("l0e", [384, 256], BF16, kind="ExternalInput").ap()
    l1o_d = nc.dram_tensor("l1o", [384, 128], BF16, kind="ExternalInput").ap()
    l1e_d = nc.dram_tensor("l1e", [128, 128], BF16, kind="ExternalInput").ap()
    outT_d = nc.dram_tensor("outT", [1024, NSH], BF16, kind="ExternalOutput").ap()

    with tile.TileContext(nc) as tc:
        _emit(tc, nc, xT_d, wt000_d, wt011_d, wt101_d, wt110_d, wt111_d,
              l0e_d, l1o_d, l1e_d, outT_d)

    nc.compile()
    return nc


def _emit(tc, nc, xT_d, wt000_d, wt011_d, wt101_d, wt110_d, wt111_d,
          l0e_d, l1o_d, l1e_d, outT_d):
    zblocks = [(i * 512, 512) for i in range(12)] + [(6144, 128)]

    # plain tensor_tensor: the only 2-tensor op with a 2x_1p uop on HW
    def vmul(eng, out, a, b):
        eng.tensor_mul(out, a, b)

    def as3(ap, Z):
        return ap.rearrange("p (j z) -> p j z", z=Z)

    def bc3(ap, Z):
        # [128, Z] slice -> broadcast [128, 3, Z] (stride 0 over j)
        return ap.rearrange("p (o z) -> p o z", o=1).broadcast_to((128, 3, Z))

    with (
        tc.tile_pool(name="wpool", bufs=1) as wpool,
        tc.tile_pool(name="xin", bufs=4) as xin,
        tc.tile_pool(name="mid", bufs=4) as mid,
        tc.tile_pool(name="oev", bufs=2) as oev,
        tc.tile_pool(name="psX", bufs=2, space="PSUM") as psX,
        tc.tile_pool(name="psY", bufs=2, space="PSUM") as psY,
    ):
        # ---- resident weights (bf16), loaded via ACT's DMA queue ---------
        def wtile(name, dram_ap, rows, cols):
            t = wpool.tile([128, cols], BF16, name=name)
            nc.scalar.dma_start(t[:, :], dram_ap[rows:rows + 128, :])
            return t

        w111 = wtile("w111", wt111_d, 0, 128)
        w011 = wtile("w011", wt011_d, 0, 256)
        w000 = [wtile(f"w000_{k}", wt000_d, 128 * k, 256) for k in range(2)]
        w101 = [wtile(f"w101_{k}", wt101_d, 128 * k, 128) for k in range(2)]
        w110 = wtile("w110", wt110_d, 0, 128)
        L1e = wtile("l1e", l1e_d, 0, 128)
        L1o = [wtile(f"l1o_{k}", l1o_d, 128 * k, 128) for k in range(3)]
        L0e = [wtile(f"l0e_{k}", l0e_d, 128 * k, 256) for k in range(3)]

        def mm(out, w, rhs, start=True, stop=True):
            nc.tensor.matmul(out, w, rhs, start=start, stop=stop)

        def ps3(name):
            return psX.tile([128, 3 * TW], F32, name=name, tag="x3")

        def ps1(name):
            return psY.tile([128, TW], F32, name=name, tag="y1")

        # ---------------- per-block pieces -------------------------------
        def loads(z0, Z):
            Z2, Z3 = 2 * Z, 3 * Z
            vcat = xin.tile([128, 3 * TW], BF16, name="vcat")
            nc.sync.dma_start(
                as3(vcat[:, :Z3], Z),
                xT_d[256:640, z0:z0 + Z].rearrange("(j p) z -> p j z", p=128))
            scat = xin.tile([128, 2 * TW], BF16, name="scat")
            nc.sync.dma_start(
                as3(scat[:, :Z2], Z),
                xT_d[0:256, z0:z0 + Z].rearrange("(m p) z -> p m z", p=128))
            return vcat, scat

        # stage-2 output groups: 1-bank PSUM tiles, evac on ACT, per-group
        # store on Sync.  Each is a generator-style callable so groups can
        # be interleaved into stage-1's natural PE stall points.
        def store3(ev, z0, Z, row0, ngrp):
            nc.sync.dma_start(
                outT_d[row0:row0 + 128 * ngrp, z0:z0 + Z]
                .rearrange("(j p) z -> p j z", p=128),
                as3(ev[:, :Z * ngrp], Z))

        def out_1o(st, j, ev):
            z0, Z, p2m, p3, p5, p1, p4 = st
            o = ps1(f"o1o_{j}")
            tp1o = [p2m[0], p2m[1], p3]
            for ci in range(3):
                mm(o[:, :Z], L1o[ci][:, :], tp1o[ci][:, j * Z:(j + 1) * Z],
                   start=(ci == 0), stop=(ci == 2))
            nc.scalar.copy(ev[:, j * Z:(j + 1) * Z], o[:, :Z])
            if j == 2:
                store3(ev, z0, Z, 256, 3)

        def out_1e(st, j, ev):
            z0, Z, p2m, p3, p5, p1, p4 = st
            o = ps1(f"o1e_{j}")
            mm(o[:, :Z], L1e[:, :], p5[:, j * Z:(j + 1) * Z])
            nc.scalar.copy(ev[:, j * Z:(j + 1) * Z], o[:, :Z])
            if j == 2:
                store3(ev, z0, Z, 640, 3)

        def out_0e(st, m, ev):
            z0, Z, p2m, p3, p5, p1, p4 = st
            o = ps1(f"o0e_{m}")
            mm(o[:, :Z], L0e[0][:, 128 * m:128 * (m + 1)], p1[:, 0:Z],
               start=True, stop=False)
            mm(o[:, :Z], L0e[1][:, 128 * m:128 * (m + 1)], p1[:, Z:2 * Z],
               start=False, stop=False)
            mm(o[:, :Z], L0e[2][:, 128 * m:128 * (m + 1)], p4[:, :Z],
               start=False, stop=True)
            nc.scalar.copy(ev[:, m * Z:(m + 1) * Z], o[:, :Z])
            if m == 1:
                store3(ev, z0, Z, 0, 2)

        def iteration(z0, Z, prev):
            """Emit stage-1 of this block, with stage-2 of the block from
            two iterations ago interleaved into PE wait windows."""
            Z2, Z3 = 2 * Z, 3 * Z
            vcat, scat = loads(z0, Z)
            if prev is not None:
                ev1o = oev.tile([128, 3 * TW], BF16, name="ev1o", tag="e3")
                ev1e = oev.tile([128, 3 * TW], BF16, name="ev1e", tag="e3")
                ev0e = oev.tile([128, 2 * TW], BF16, name="ev0e", tag="e2")

            # E matmuls; stored [E2|E0|E1] so the p5 cross products are
            # contiguous-range elementwise ops
            Ecat = ps3("Ecat")
            mm(Ecat[:, 0:Z], w111[:, :], vcat[:, Z2:Z3])
            mm(Ecat[:, Z:Z2], w111[:, :], vcat[:, 0:Z])
            mm(Ecat[:, Z2:Z3], w111[:, :], vcat[:, Z:Z2])
            Es = mid.tile([128, 3 * TW], BF16, name="Es")
            nc.scalar.copy(Es[:, :Z3], Ecat[:, :Z3])

            # b matmuls for m=0 (p2 mul emitted right after, to free bank)
            bm0 = ps3("bm0")
            for j in range(3):
                mm(bm0[:, j * Z:(j + 1) * Z], w011[:, 0:128],
                   vcat[:, j * Z:(j + 1) * Z])
            p2m = [mid.tile([128, 3 * TW], BF16, name="p2m0")]
            vmul(nc.vector, as3(p2m[0][:, :Z3], Z), as3(bm0[:, :Z3], Z),
                 bc3(scat[:, 0:Z], Z))

            # fill PE while Es/p2m0 drain: previous block's 1o outputs
            if prev is not None:
                for j in range(3):
                    out_1o(prev, j, ev1o)

            bm1 = ps3("bm1")
            for j in range(3):
                mm(bm1[:, j * Z:(j + 1) * Z], w011[:, 128:256],
                   vcat[:, j * Z:(j + 1) * Z])
            p2m.append(mid.tile([128, 3 * TW], BF16, name="p2m1"))
            vmul(nc.vector, as3(p2m[1][:, :Z3], Z), as3(bm1[:, :Z3], Z),
                 bc3(scat[:, Z:Z2], Z))

            # acM = [a0|a1|c]
            acM = ps3("acM")
            for m in range(2):
                mm(acM[:, m * Z:(m + 1) * Z],
                   w000[0][:, 128 * m:128 * (m + 1)], scat[:, 0:Z],
                   start=True, stop=False)
                mm(acM[:, m * Z:(m + 1) * Z],
                   w000[1][:, 128 * m:128 * (m + 1)], scat[:, Z:Z2],
                   start=False, stop=True)
            mm(acM[:, Z2:Z3], w101[0][:, :], scat[:, 0:Z],
               start=True, stop=False)
            mm(acM[:, Z2:Z3], w101[1][:, :], scat[:, Z:Z2],
               start=False, stop=True)
            p1 = mid.tile([128, 2 * TW], BF16, name="p1")
            vmul(nc.vector, p1[:, :Z2], acM[:, 0:Z2], scat[:, 0:Z2])
            cs = mid.tile([128, TW], BF16, name="cs")
            nc.scalar.copy(cs[:, :Z], acM[:, Z2:Z3])

            # previous block's 1e outputs
            if prev is not None:
                for j in range(3):
                    out_1e(prev, j, ev1e)

            # d matmuls
            dcat = ps3("dcat")
            for j in range(3):
                mm(dcat[:, j * Z:(j + 1) * Z], w110[:, :],
                   vcat[:, j * Z:(j + 1) * Z])
            ds = mid.tile([128, 3 * TW], BF16, name="ds")
            nc.scalar.copy(ds[:, :Z3], dcat[:, :Z3])

            # previous block's 0e outputs
            if prev is not None:
                for m in range(2):
                    out_0e(prev, m, ev0e)

            # SBUF-side elementwise (DVE 2x_1p / GpSimd)
            ta = mid.tile([128, 3 * TW], BF16, name="ta")
            vmul(nc.vector, ta[:, 0:Z2], vcat[:, Z:Z3], Es[:, 0:Z2])
            vmul(nc.gpsimd, ta[:, Z2:Z3], vcat[:, 0:Z], Es[:, Z2:Z3])
            tb = mid.tile([128, 3 * TW], BF16, name="tb")
            vmul(nc.gpsimd, tb[:, 0:Z], vcat[:, Z2:Z3], Es[:, Z2:Z3])
            vmul(nc.vector, tb[:, Z:Z3], vcat[:, 0:Z2], Es[:, 0:Z2])
            p5 = mid.tile([128, 3 * TW], BF16, name="p5")
            nc.vector.tensor_sub(p5[:, :Z3], ta[:, :Z3], tb[:, :Z3])

            p3 = mid.tile([128, 3 * TW], BF16, name="p3")
            vmul(nc.vector, as3(p3[:, :Z3], Z), as3(vcat[:, :Z3], Z),
                 bc3(cs[:, 0:Z], Z))
            t4 = mid.tile([128, 3 * TW], BF16, name="t4")
            vmul(nc.vector, t4[:, :Z3], ds[:, :Z3], vcat[:, :Z3])
            p4 = mid.tile([128, TW], BF16, name="p4")
            nc.gpsimd.tensor_add(p4[:, :Z], t4[:, 0:Z], t4[:, Z:Z2])
            nc.gpsimd.tensor_add(p4[:, :Z], p4[:, :Z], t4[:, Z2:Z3])

            return (z0, Z, p2m, p3, p5, p1, p4)

        hist = []
        for (z0, Z) in zblocks:
            prev = hist.pop(0) if len(hist) >= 2 else None
            hist.append(iteration(z0, Z, prev))
        for st in hist:
            ev1o = oev.tile([128, 3 * TW], BF16, name="ev1o", tag="e3")
            ev1e = oev.tile([128, 3 * TW], BF16, name="ev1e", tag="e3")
            ev0e = oev.tile([128, 2 * TW], BF16, name="ev0e", tag="e2")
            for j in range(3):
                out_1o(st, j, ev1o)
            for j in range(3):
                out_1e(st, j, ev1e)
            for m in range(2):
                out_0e(st, m, ev0e)


def _prep_inputs(node_feat, w_00_0, w_01_1, w_10_1, w_11_0, w_11_1,
                 W_0e, W_1o, W_1e):
    import ml_dtypes
    ndt = ml_dtypes.bfloat16
    weights = {
        "wt000": np.ascontiguousarray((C_000 * w_00_0).T).astype(ndt),
        "wt011": np.ascontiguousarray((C_011 * w_01_1).T).astype(ndt),
        "wt101": np.ascontiguousarray((C_101 * w_10_1).T).astype(ndt),
        "wt110": np.ascontiguousarray((C_110 * w_11_0).T).astype(ndt),
        "wt111": np.ascontiguousarray((C_111 * w_11_1).T).astype(ndt),
        "l0e": np.ascontiguousarray(W_0e / np.sqrt(384.0)).astype(ndt),
        "l1o": np.ascontiguousarray(W_1o / np.sqrt(384.0)).astype(ndt),
        "l1e": np.ascontiguousarray(W_1e / np.sqrt(128.0)).astype(ndt),
    }
    feat = np.asarray(node_feat, dtype=np.float32).reshape(N_CORES, NS, 640)
    in_maps = []
    for i in range(N_CORES):
        blk = feat[i]
        xT = np.zeros((640, NSH), ndt)
        xT[:256, :NS] = blk[:, :256].T.astype(ndt)
        vv = blk[:, 256:].reshape(NS, 128, 3)
        xT[256:, :NS] = vv.transpose(2, 1, 0).reshape(384, NS).astype(ndt)
        in_maps.append({"xT": xT, **weights})
    return in_maps


def _gather(results):
    out = np.empty((N_NODES, 1024), np.float32)
    for i in range(N_CORES):
        oT = np.asarray(results[i]["outT"]).astype(np.float32,
                                                   copy=False)[:, :NS]
        blk = out[i * NS:(i + 1) * NS]
        blk[:, :256] = oT[:256].T
        blk[:, 256:640] = oT[256:640].reshape(3, 128, NS).transpose(2, 1, 0) \
            .reshape(NS, 384)
        blk[:, 640:] = oT[640:].reshape(3, 128, NS).transpose(2, 1, 0) \
            .reshape(NS, 384)
    return out


def kernel(node_feat, w_00_0, w_01_1, w_10_1, w_11_0, w_11_1,
           W_0e, W_1o, W_1e, _trace=False):
    if "v1" not in _CACHE:
        _CACHE["v1"] = _build_program()
    nc = _CACHE["v1"]
    in_maps = _prep_inputs(node_feat, w_00_0, w_01_1, w_10_1, w_11_0,
                           w_11_1, W_0e, W_1o, W_1e)
    res = run_bass_kernel_spmd(nc, in_maps, core_ids=list(range(N_CORES)),
                               trace=_trace)
    out = _gather(res.results)
    if _trace:
        return out, res
    return out


# revision 14
# speedup vs baseline: 1.5692x; 1.0180x over previous
"""Trainium2 Bass kernel for nn_NodePreTrans (e3nn tensor product + linear).

Data-parallel over nodes: 50000 rows sharded 8 ways (6250/core, padded to
6272).  Channel-major layout; bf16 I/O + bf16 matmuls (f32 PSUM), merged
wide elementwise ops on DVE (scalar_tensor_tensor, 4x mode for SBUF bf16)
and merged PSUM evacuations on ACT.  PSUM: one 3-bank rotating pool (x2)
plus a 2-bank pool = 8 banks.
"""

import sys

sys.path.insert(0, "/opt/trn_rl_repo")

import numpy as np

import concourse.bacc as bacc
import concourse.bass as bass
import concourse.mybir as mybir
import concourse.tile as tile
from concourse.bass_utils import run_bass_kernel_spmd

N_NODES = 50000
N_CORES = 8
NS = N_NODES // N_CORES          # 6250 real nodes per core
NSH = 6272                       # padded (12*512 + 128)
TW = 512                         # PSUM bank width in f32

C_000 = 1.0 / np.sqrt(256.0)
C_011 = 1.0 / np.sqrt(128.0)
C_101 = 1.0 / np.sqrt(256.0)
C_110 = 1.0 / np.sqrt(384.0)
C_111 = 1.0 / 16.0

F32 = mybir.dt.float32
BF16 = mybir.dt.bfloat16
AOP = mybir.AluOpType

_CACHE = {}


def _build_program():
    nc = bacc.Bacc("TRN2", target_bir_lowering=False, debug=False,
                   num_devices=N_CORES)

    xT_d = nc.dram_tensor("xT", [640, NSH], BF16, kind="ExternalInput").ap()
    wt000_d = nc.dram_tensor("wt000", [256, 256], BF16, kind="ExternalInput").ap()
    wt011_d = nc.dram_tensor("wt011", [128, 256], BF16, kind="ExternalInput").ap()
    wt101_d = nc.dram_tensor("wt101", [256, 128], BF16, kind="ExternalInput").ap()
    wt110_d = nc.dram_tensor("wt110", [128, 128], BF16, kind="ExternalInput").ap()
    wt111_d = nc.dram_tensor("wt111", [128, 128], BF16, kind="ExternalInput").ap()
    l0e_d = nc.dram_tensor("l0e", [384, 256], BF16, kind="ExternalInput").ap()
    l1o_d = nc.dram_tensor("l1o", [384, 128], BF16, kind="ExternalInput").ap()
    l1e_d = nc.dram_tensor("l1e", [128, 128], BF16, kind="ExternalInput").ap()
    outT_d = nc.dram_tensor("outT", [1024, NSH], BF16, kind="ExternalOutput").ap()

    with tile.TileContext(nc) as tc:
        _emit(tc, nc, xT_d, wt000_d, wt011_d, wt101_d, wt110_d, wt111_d,
              l0e_d, l1o_d, l1e_d, outT_d)

    nc.compile()
    return nc


def _emit(tc, nc, xT_d, wt000_d, wt011_d, wt101_d, wt110_d, wt111_d,
          l0e_d, l1o_d, l1e_d, outT_d):
    zblocks = [(i * 512, 512) for i in range(12)] + [(6144, 128)]

    # plain tensor_tensor: the only 2-tensor op with a 2x_1p uop on HW
    def vmul(eng, out, a, b):
        eng.tensor_mul(out, a, b)

    def as3(ap, Z):
        return ap.rearrange("p (j z) -> p j z", z=Z)

    def bc3(ap, Z):
        # [128, Z] slice -> broadcast [128, 3, Z] (stride 0 over j)
        return ap.rearrange("p (o z) -> p o z", o=1).broadcast_to((128, 3, Z))

    with (
        tc.tile_pool(name="wpool", bufs=1) as wpool,
        tc.tile_pool(name="xin", bufs=4) as xin,
        tc.tile_pool(name="mid", bufs=4) as mid,
        tc.tile_pool(name="oev", bufs=2) as oev,
        tc.tile_pool(name="psX", bufs=2, space="PSUM") as psX,
        tc.tile_pool(name="psY", bufs=2, space="PSUM") as psY,
    ):
        # ---- resident weights (bf16), loaded via ACT's DMA queue ---------
        def wtile(name, dram_ap, rows, cols):
            t = wpool.tile([128, cols], BF16, name=name)
            nc.scalar.dma_start(t[:, :], dram_ap[rows:rows + 128, :])
            return t

        w111 = wtile("w111", wt111_d, 0, 128)
        w011 = wtile("w011", wt011_d, 0, 256)
        w000 = [wtile(f"w000_{k}", wt000_d, 128 * k, 256) for k in range(2)]
        w101 = [wtile(f"w101_{k}", wt101_d, 128 * k, 128) for k in range(2)]
        w110 = wtile("w110", wt110_d, 0, 128)
        L1e = wtile("l1e", l1e_d, 0, 128)
        L1o = [wtile(f"l1o_{k}", l1o_d, 128 * k, 128) for k in range(3)]
        L0e = [wtile(f"l0e_{k}", l0e_d, 128 * k, 256) for k in range(3)]

        def mm(out, w, rhs, start=True, stop=True):
            nc.tensor.matmul(out, w, rhs, start=start, stop=stop)

        def ps3(name):
            return psX.tile([128, 3 * TW], F32, name=name, tag="x3")

        def ps1(name):
            return psY.tile([128, TW], F32, name=name, tag="y1")

        # ---------------- per-block pieces -------------------------------
        def loads(z0, Z):
            Z2, Z3 = 2 * Z, 3 * Z
            vcat = xin.tile([128, 3 * TW], BF16, name="vcat")
            nc.sync.dma_start(
                as3(vcat[:, :Z3], Z),
                xT_d[256:640, z0:z0 + Z].rearrange("(j p) z -> p j z", p=128))
            scat = xin.tile([128, 2 * TW], BF16, name="scat")
            nc.sync.dma_start(
                as3(scat[:, :Z2], Z),
                xT_d[0:256, z0:z0 + Z].rearrange("(m p) z -> p m z", p=128))
            return vcat, scat

        # stage-2 output groups: 1-bank PSUM tiles, evac on ACT, per-group
        # store on Sync.  Each is a generator-style callable so groups can
        # be interleaved into stage-1's natural PE stall points.
        def store3(ev, z0, Z, row0, ngrp):
            nc.sync.dma_start(
                outT_d[row0:row0 + 128 * ngrp, z0:z0 + Z]
                .rearrange("(j p) z -> p j z", p=128),
                as3(ev[:, :Z * ngrp], Z))

        def out_1o(st, j, ev):
            z0, Z, p2m, p3, p5, p1, p4 = st
            o = ps1(f"o1o_{j}")
            tp1o = [p2m[0], p2m[1], p3]
            for ci in range(3):
                mm(o[:, :Z], L1o[ci][:, :], tp1o[ci][:, j * Z:(j + 1) * Z],
                   start=(ci == 0), stop=(ci == 2))
            nc.scalar.copy(ev[:, j * Z:(j + 1) * Z], o[:, :Z])
            if j == 2:
                store3(ev, z0, Z, 256, 3)

        def out_1e(st, j, ev):
            z0, Z, p2m, p3, p5, p1, p4 = st
            o = ps1(f"o1e_{j}")
            mm(o[:, :Z], L1e[:, :], p5[:, j * Z:(j + 1) * Z])
            nc.scalar.copy(ev[:, j * Z:(j + 1) * Z], o[:, :Z])
            if j == 2:
                store3(ev, z0, Z, 640, 3)

        def out_0e(st, m, ev):
            z0, Z, p2m, p3, p5, p1, p4 = st
            o = ps1(f"o0e_{m}")
            mm(o[:, :Z], L0e[0][:, 128 * m:128 * (m + 1)], p1[:, 0:Z],
               start=True, stop=False)
            mm(o[:, :Z], L0e[1][:, 128 * m:128 * (m + 1)], p1[:, Z:2 * Z],
               start=False, stop=False)
            mm(o[:, :Z], L0e[2][:, 128 * m:128 * (m + 1)], p4[:, :Z],
               start=False, stop=True)
            nc.scalar.copy(ev[:, m * Z:(m + 1) * Z], o[:, :Z])
            if m == 1:
                store3(ev, z0, Z, 0, 2)

        def iteration(z0, Z, prev):
            """Emit stage-1 of this block, with stage-2 of the block from
            two iterations ago interleaved into PE wait windows."""
            Z2, Z3 = 2 * Z, 3 * Z
            vcat, scat = loads(z0, Z)
            if prev is not None:
                ev1o = oev.tile([128, 3 * TW], BF16, name="ev1o", tag="e3")
                ev1e = oev.tile([128, 3 * TW], BF16, name="ev1e", tag="e3")
                ev0e = oev.tile([128, 2 * TW], BF16, name="ev0e", tag="e2")

            # E matmuls; stored [E2|E0|E1] so the p5 cross products are
            # contiguous-range elementwise ops
            Ecat = ps3("Ecat")
            mm(Ecat[:, 0:Z], w111[:, :], vcat[:, Z2:Z3])
            mm(Ecat[:, Z:Z2], w111[:, :], vcat[:, 0:Z])
            mm(Ecat[:, Z2:Z3], w111[:, :], vcat[:, Z:Z2])
            Es = mid.tile([128, 3 * TW], BF16, name="Es")
            nc.scalar.copy(Es[:, :Z3], Ecat[:, :Z3])

            # b matmuls for m=0 (p2 mul emitted right after, to free bank)
            bm0 = ps3("bm0")
            for j in range(3):
                mm(bm0[:, j * Z:(j + 1) * Z], w011[:, 0:128],
                   vcat[:, j * Z:(j + 1) * Z])
            p2m = [mid.tile([128, 3 * TW], BF16, name="p2m0")]
            vmul(nc.vector, as3(p2m[0][:, :Z3], Z), as3(bm0[:, :Z3], Z),
                 bc3(scat[:, 0:Z], Z))

            # fill PE while Es/p2m0 drain: previous block's outputs
            if prev is not None:
                out_1o(prev, 0, ev1o)
                out_1e(prev, 0, ev1e)

            bm1 = ps3("bm1")
            for j in range(3):
                mm(bm1[:, j * Z:(j + 1) * Z], w011[:, 128:256],
                   vcat[:, j * Z:(j + 1) * Z])
            p2m.append(mid.tile([128, 3 * TW], BF16, name="p2m1"))
            vmul(nc.vector, as3(p2m[1][:, :Z3], Z), as3(bm1[:, :Z3], Z),
                 bc3(scat[:, Z:Z2], Z))

            if prev is not None:
                out_1o(prev, 1, ev1o)
                out_1e(prev, 1, ev1e)

            # acM = [a0|a1|c]
            acM = ps3("acM")
            for m in range(2):
                mm(acM[:, m * Z:(m + 1) * Z],
                   w000[0][:, 128 * m:128 * (m + 1)], scat[:, 0:Z],
                   start=True, stop=False)
                mm(acM[:, m * Z:(m + 1) * Z],
                   w000[1][:, 128 * m:128 * (m + 1)], scat[:, Z:Z2],
                   start=False, stop=True)
            mm(acM[:, Z2:Z3], w101[0][:, :], scat[:, 0:Z],
               start=True, stop=False)
            mm(acM[:, Z2:Z3], w101[1][:, :], scat[:, Z:Z2],
               start=False, stop=True)
            p1 = mid.tile([128, 2 * TW], BF16, name="p1")
            vmul(nc.vector, p1[:, :Z2], acM[:, 0:Z2], scat[:, 0:Z2])
            cs = mid.tile([128, TW], BF16, name="cs")
            nc.scalar.copy(cs[:, :Z], acM[:, Z2:Z3])

            if prev is not None:
                out_1o(prev, 2, ev1o)
                out_1e(prev, 2, ev1e)

            # d matmuls
            dcat = ps3("dcat")
            for j in range(3):
                mm(dcat[:, j * Z:(j + 1) * Z], w110[:, :],
                   vcat[:, j * Z:(j + 1) * Z])
            ds = mid.tile([128, 3 * TW], BF16, name="ds")
            nc.scalar.copy(ds[:, :Z3], dcat[:, :Z3])

            if prev is not None:
                out_0e(prev, 0, ev0e)
                out_0e(prev, 1, ev0e)

            # SBUF-side elementwise (DVE 2x_1p / GpSimd)
            ta = mid.tile([128, 3 * TW], BF16, name="ta")
            vmul(nc.vector, ta[:, 0:Z2], vcat[:, Z:Z3], Es[:, 0:Z2])
            vmul(nc.gpsimd, ta[:, Z2:Z3], vcat[:, 0:Z], Es[:, Z2:Z3])
            tb = mid.tile([128, 3 * TW], BF16, name="tb")
            vmul(nc.gpsimd, tb[:, 0:Z], vcat[:, Z2:Z3], Es[:, Z2:Z3])
            vmul(nc.vector, tb[:, Z:Z3], vcat[:, 0:Z2], Es[:, 0:Z2])
            p5 = mid.tile([128, 3 * TW], BF16, name="p5")
            nc.vector.tensor_sub(p5[:, :Z3], ta[:, :Z3], tb[:, :Z3])

            p3 = mid.tile([128, 3 * TW], BF16, name="p3")
            vmul(nc.vector, as3(p3[:, :Z3], Z), as3(vcat[:, :Z3], Z),
                 bc3(cs[:, 0:Z], Z))
            t4 = mid.tile([128, 3 * TW], BF16, name="t4")
            vmul(nc.vector, t4[:, :Z3], ds[:, :Z3], vcat[:, :Z3])
            p4 = mid.tile([128, TW], BF16, name="p4")
            nc.gpsimd.tensor_add(p4[:, :Z], t4[:, 0:Z], t4[:, Z:Z2])
            nc.gpsimd.tensor_add(p4[:, :Z], p4[:, :Z], t4[:, Z2:Z3])

            return (z0, Z, p2m, p3, p5, p1, p4)

        hist = []
        for (z0, Z) in zblocks:
            prev = hist.pop(0) if len(hist) >= 2 else None
            hist.append(iteration(z0, Z, prev))
        for st in hist:
            ev1o = oev.tile([128, 3 * TW], BF16, name="ev1o", tag="e3")
            ev1e = oev.tile([128, 3 * TW], BF16, name="ev1e", tag="e3")
            ev0e = oev.tile([128, 2 * TW], BF16, name="ev0e", tag="e2")
            for j in range(3):
                out_1o(st, j, ev1o)
            for j in range(3):
                out_1e(st, j, ev1e)
            for m in range(2):
                out_0e(st, m, ev0e)


def _prep_inputs(node_feat, w_00_0, w_01_1, w_10_1, w_11_0, w_11_1,
                 W_0e, W_1o, W_1e):
    import ml_dtypes
    ndt = ml_dtypes.bfloat16
    weights = {
        "wt000": np.ascontiguousarray((C_000 * w_00_0).T).astype(ndt),
        "wt011": np.ascontiguousarray((C_011 * w_01_1).T).astype(ndt),
        "wt101": np.ascontiguousarray((C_101 * w_10_1).T).astype(ndt),
        "wt110": np.ascontiguousarray((C_110 * w_11_0).T).astype(ndt),
        "wt111": np.ascontiguousarray((C_111 * w_11_1).T).astype(ndt),
        "l0e": np.ascontiguousarray(W_0e / np.sqrt(384.0)).astype(ndt),
        "l1o": np.ascontiguousarray(W_1o / np.sqrt(384.0)).astype(ndt),
        "l1e": np.ascontiguousarray(W_1e / np.sqrt(128.0)).astype(ndt),
    }
    feat = np.asarray(node_feat, dtype=np.float32).reshape(N_CORES, NS, 640)
    in_maps = []
    for i in range(N_CORES):
        blk = feat[i]
        xT = np.zeros((640, NSH), ndt)
        xT[:256, :NS] = blk[:, :256].T.astype(ndt)
        vv = blk[:, 256:].reshape(NS, 128, 3)
        xT[256:, :NS] = vv.transpose(2, 1, 0).reshape(384, NS).astype(ndt)
        in_maps.append({"xT": xT, **weights})
    return in_maps


def _gather(results):
    out = np.empty((N_NODES, 1024), np.float32)
    for i in range(N_CORES):
        oT = np.asarray(results[i]["outT"]).astype(np.float32,
                                                   copy=False)[:, :NS]
        blk = out[i * NS:(i + 1) * NS]
        blk[:, :256] = oT[:256].T
        blk[:, 256:640] = oT[256:640].reshape(3, 128, NS).transpose(2, 1, 0) \
            .reshape(NS, 384)
        blk[:, 640:] = oT[640:].reshape(3, 128, NS).transpose(2, 1, 0) \
            .reshape(NS, 384)
    return out


def kernel(node_feat, w_00_0, w_01_1, w_10_1, w_11_0, w_11_1,
           W_0e, W_1o, W_1e, _trace=False):
    if "v1" not in _CACHE:
        _CACHE["v1"] = _build_program()
    nc = _CACHE["v1"]
    in_maps = _prep_inputs(node_feat, w_00_0, w_01_1, w_10_1, w_11_0,
                           w_11_1, W_0e, W_1o, W_1e)
    res = run_bass_kernel_spmd(nc, in_maps, core_ids=list(range(N_CORES)),
                               trace=_trace)
    out = _gather(res.results)
    if _trace:
        return out, res
    return out


# revision 29
# speedup vs baseline: 1.6299x; 1.0386x over previous
"""Trainium2 Bass kernel for nn_NodePreTrans (e3nn tensor product + linear).

Data-parallel over nodes: 50000 rows sharded 8 ways (6250/core, padded to
6272).  Channel-major layout; bf16 I/O + bf16 matmuls (f32 PSUM).  Merged
wide elementwise ops on DVE (tensor_tensor 2x_1p for SBUF bf16; PSUM
operands run 1x), merged PSUM evacuations on ACT, FD=512 SBUF muls/adds on
GpSimd.  PSUM: stage-1 in a 3-bank x2 rotating pool, stage-2 in 1-bank x2
tiles (8 banks total).  Two-deep software pipeline: stage-2 of block i-2
is interleaved into stage-1 of block i at PE stall points.
"""

import sys

sys.path.insert(0, "/opt/trn_rl_repo")

import numpy as np

import concourse.bacc as bacc
import concourse.bass as bass
import concourse.mybir as mybir
import concourse.tile as tile
from concourse.bass_utils import run_bass_kernel_spmd

N_NODES = 50000
N_CORES = 8
NS = N_NODES // N_CORES          # 6250 real nodes per core
NSH = 6272                       # padded (12*512 + 128)
TW = 512                         # PSUM bank width in f32

C_000 = 1.0 / np.sqrt(256.0)
C_011 = 1.0 / np.sqrt(128.0)
C_101 = 1.0 / np.sqrt(256.0)
C_110 = 1.0 / np.sqrt(384.0)
C_111 = 1.0 / 16.0

F32 = mybir.dt.float32
BF16 = mybir.dt.bfloat16
AOP = mybir.AluOpType

_CACHE = {}


def _build_program():
    nc = bacc.Bacc("TRN2", target_bir_lowering=False, debug=False,
                   num_devices=N_CORES)

    xT_d = nc.dram_tensor("xT", [640, NSH], BF16, kind="ExternalInput").ap()
    wt000_d = nc.dram_tensor("wt000", [256, 256], BF16, kind="ExternalInput").ap()
    wt011_d = nc.dram_tensor("wt011", [128, 256], BF16, kind="ExternalInput").ap()
    wt101_d = nc.dram_tensor("wt101", [256, 128], BF16, kind="ExternalInput").ap()
    wt110_d = nc.dram_tensor("wt110", [128, 128], BF16, kind="ExternalInput").ap()
    wt111_d = nc.dram_tensor("wt111", [128, 128], BF16, kind="ExternalInput").ap()
    l0e_d = nc.dram_tensor("l0e", [384, 256], BF16, kind="ExternalInput").ap()
    l1o_d = nc.dram_tensor("l1o", [384, 128], BF16, kind="ExternalInput").ap()
    l1e_d = nc.dram_tensor("l1e", [128, 128], BF16, kind="ExternalInput").ap()
    l1en_d = nc.dram_tensor("l1en", [128, 128], BF16, kind="ExternalInput").ap()
    outT_d = nc.dram_tensor("outT", [1024, NSH], BF16, kind="ExternalOutput").ap()

    with tile.TileContext(nc) as tc:
        _emit(tc, nc, xT_d, wt000_d, wt011_d, wt101_d, wt110_d, wt111_d,
              l0e_d, l1o_d, l1e_d, l1en_d, outT_d)

    nc.compile()
    return nc


def _emit(tc, nc, xT_d, wt000_d, wt011_d, wt101_d, wt110_d, wt111_d,
          l0e_d, l1o_d, l1e_d, l1en_d, outT_d):
    zblocks = [(i * 512, 512) for i in range(12)] + [(6144, 128)]

    # plain tensor_tensor: the only 2-tensor op with a 2x_1p uop on HW
    def vmul(eng, out, a, b):
        eng.tensor_mul(out, a, b)

    def as3(ap, Z):
        return ap.rearrange("p (j z) -> p j z", z=Z)

    def bc3(ap, Z):
        # [128, Z] slice -> broadcast [128, 3, Z] (stride 0 over j)
        return ap.rearrange("p (o z) -> p o z", o=1).broadcast_to((128, 3, Z))

    with (
        tc.tile_pool(name="wpool", bufs=1) as wpool,
        tc.tile_pool(name="xin", bufs=4) as xin,
        tc.tile_pool(name="mid", bufs=4) as mid,
        tc.tile_pool(name="oev", bufs=2) as oev,
        tc.tile_pool(name="psX", bufs=2, space="PSUM") as psX,
        tc.tile_pool(name="psY", bufs=2, space="PSUM") as psY,
    ):
        # ---- resident weights (bf16) ------------------------------------
        # issue on GpSimd's queue: ACT's queue must stay free for block 1's
        # Es evac, and Sync's for the x loads
        def wtile(name, dram_ap, rows, cols):
            t = wpool.tile([128, cols], BF16, name=name)
            nc.gpsimd.dma_start(t[:, :], dram_ap[rows:rows + 128, :])
            return t

        w111 = wtile("w111", wt111_d, 0, 128)
        w011 = wtile("w011", wt011_d, 0, 256)
        w000 = [wtile(f"w000_{k}", wt000_d, 128 * k, 256) for k in range(2)]
        w101 = [wtile(f"w101_{k}", wt101_d, 128 * k, 128) for k in range(2)]
        w110 = wtile("w110", wt110_d, 0, 128)
        L1e = wtile("l1e", l1e_d, 0, 128)
        L1en = wtile("l1en", l1en_d, 0, 128)
        L1o = [wtile(f"l1o_{k}", l1o_d, 128 * k, 128) for k in range(3)]
        L0e = [wtile(f"l0e_{k}", l0e_d, 128 * k, 256) for k in range(3)]

        def mm(out, w, rhs, start=True, stop=True):
            nc.tensor.matmul(out, w, rhs, start=start, stop=stop)

        def ps3(name):
            return psX.tile([128, 3 * TW], F32, name=name, tag="x3")

        def ps1(name):
            return psY.tile([128, TW], F32, name=name, tag="y1")

        # ---------------- per-block pieces -------------------------------
        def loads(z0, Z):
            Z2, Z3 = 2 * Z, 3 * Z
            vcat = xin.tile([128, 3 * TW], BF16, name="vcat")
            nc.sync.dma_start(
                as3(vcat[:, :Z3], Z),
                xT_d[256:640, z0:z0 + Z].rearrange("(j p) z -> p j z", p=128))
            scat = xin.tile([128, 2 * TW], BF16, name="scat")
            nc.sync.dma_start(
                as3(scat[:, :Z2], Z),
                xT_d[0:256, z0:z0 + Z].rearrange("(m p) z -> p m z", p=128))
            return vcat, scat

        # stage-2 output groups: 1-bank PSUM tiles, evac on ACT, per-group
        # store on Sync.  Each is a generator-style callable so groups can
        # be interleaved into stage-1's natural PE stall points.
        def store3(ev, z0, Z, row0, ngrp):
            nc.sync.dma_start(
                outT_d[row0:row0 + 128 * ngrp, z0:z0 + Z]
                .rearrange("(j p) z -> p j z", p=128),
                as3(ev[:, :Z * ngrp], Z))

        def out_1o(st, j, ev):
            z0, Z, p2m, p3, p5, p1, p4 = st
            o = ps1(f"o1o_{j}")
            tp1o = [p2m[0], p2m[1], p3]
            for ci in range(3):
                mm(o[:, :Z], L1o[ci][:, :], tp1o[ci][:, j * Z:(j + 1) * Z],
                   start=(ci == 0), stop=(ci == 2))
            nc.scalar.copy(ev[:, j * Z:(j + 1) * Z], o[:, :Z])
            if j == 2:
                store3(ev, z0, Z, 256, 3)

        def out_1e(st, j, ev):
            z0, Z, p2m, p3, p5, p1, p4 = st
            o = ps1(f"o1e_{j}")
            mm(o[:, :Z], L1e[:, :], p5[:, j * Z:(j + 1) * Z])
            nc.scalar.copy(ev[:, j * Z:(j + 1) * Z], o[:, :Z])
            if j == 2:
                store3(ev, z0, Z, 640, 3)

        def out_0e(st, m, ev):
            z0, Z, p2m, p3, p5, p1, p4 = st
            o = ps1(f"o0e_{m}")
            mm(o[:, :Z], L0e[0][:, 128 * m:128 * (m + 1)], p1[:, 0:Z],
               start=True, stop=False)
            mm(o[:, :Z], L0e[1][:, 128 * m:128 * (m + 1)], p1[:, Z:2 * Z],
               start=False, stop=False)
            mm(o[:, :Z], L0e[2][:, 128 * m:128 * (m + 1)], p4[:, :Z],
               start=False, stop=True)
            nc.scalar.copy(ev[:, m * Z:(m + 1) * Z], o[:, :Z])
            if m == 1:
                store3(ev, z0, Z, 0, 2)

        def iteration(z0, Z, prev):
            """Emit stage-1 of this block, with stage-2 of the block from
            two iterations ago interleaved into PE wait windows."""
            Z2, Z3 = 2 * Z, 3 * Z
            vcat, scat = loads(z0, Z)
            if prev is not None:
                ev1o = oev.tile([128, 3 * TW], BF16, name="ev1o", tag="e3")
                ev1e = oev.tile([128, 3 * TW], BF16, name="ev1e", tag="e3")
                ev0e = oev.tile([128, 2 * TW], BF16, name="ev0e", tag="e2")

            # E matmuls; stored [E2|E0|E1] so the p5 cross products are
            # contiguous-range elementwise ops
            Ecat = ps3("Ecat")
            mm(Ecat[:, 0:Z], w111[:, :], vcat[:, Z2:Z3])
            mm(Ecat[:, Z:Z2], w111[:, :], vcat[:, 0:Z])
            mm(Ecat[:, Z2:Z3], w111[:, :], vcat[:, Z:Z2])
            Es = mid.tile([128, 3 * TW], BF16, name="Es")
            nc.scalar.copy(Es[:, :Z3], Ecat[:, :Z3])

            # acM = [a0|a1|c]
            acM = ps3("acM")
            for m in range(2):
                mm(acM[:, m * Z:(m + 1) * Z],
                   w000[0][:, 128 * m:128 * (m + 1)], scat[:, 0:Z],
                   start=True, stop=False)
                mm(acM[:, m * Z:(m + 1) * Z],
                   w000[1][:, 128 * m:128 * (m + 1)], scat[:, Z:Z2],
                   start=False, stop=True)
            mm(acM[:, Z2:Z3], w101[0][:, :], scat[:, 0:Z],
               start=True, stop=False)
            mm(acM[:, Z2:Z3], w101[1][:, :], scat[:, Z:Z2],
               start=False, stop=True)
            p1 = mid.tile([128, 2 * TW], BF16, name="p1")
            vmul(nc.vector, p1[:, :Z2], acM[:, 0:Z2], scat[:, 0:Z2])
            cs = mid.tile([128, TW], BF16, name="cs")
            nc.scalar.copy(cs[:, :Z], acM[:, Z2:Z3])

            if prev is not None:
                out_1o(prev, 0, ev1o)
                out_1e(prev, 0, ev1e)

            # b matmuls for m=0 (p2 mul emitted right after, to free bank)
            bm0 = ps3("bm0")
            for j in range(3):
                mm(bm0[:, j * Z:(j + 1) * Z], w011[:, 0:128],
                   vcat[:, j * Z:(j + 1) * Z])
            p2m = [mid.tile([128, 3 * TW], BF16, name="p2m0")]
            vmul(nc.vector, as3(p2m[0][:, :Z3], Z), as3(bm0[:, :Z3], Z),
                 bc3(scat[:, 0:Z], Z))

            if prev is not None:
                out_1o(prev, 1, ev1o)
                out_1e(prev, 1, ev1e)

            bm1 = ps3("bm1")
            for j in range(3):
                mm(bm1[:, j * Z:(j + 1) * Z], w011[:, 128:256],
                   vcat[:, j * Z:(j + 1) * Z])
            p2m.append(mid.tile([128, 3 * TW], BF16, name="p2m1"))
            vmul(nc.vector, as3(p2m[1][:, :Z3], Z), as3(bm1[:, :Z3], Z),
                 bc3(scat[:, Z:Z2], Z))

            if prev is not None:
                out_1o(prev, 2, ev1o)
                out_1e(prev, 2, ev1e)

            # d matmuls
            dcat = ps3("dcat")
            for j in range(3):
                mm(dcat[:, j * Z:(j + 1) * Z], w110[:, :],
                   vcat[:, j * Z:(j + 1) * Z])
            ds = mid.tile([128, 3 * TW], BF16, name="ds")
            nc.scalar.copy(ds[:, :Z3], dcat[:, :Z3])

            if prev is not None:
                out_0e(prev, 0, ev0e)
                out_0e(prev, 1, ev0e)

            # SBUF-side elementwise (DVE 2x_1p / GpSimd)
            ta = mid.tile([128, 3 * TW], BF16, name="ta")
            vmul(nc.vector, ta[:, 0:Z2], vcat[:, Z:Z3], Es[:, 0:Z2])
            vmul(nc.gpsimd, ta[:, Z2:Z3], vcat[:, 0:Z], Es[:, Z2:Z3])
            tb = mid.tile([128, 3 * TW], BF16, name="tb")
            vmul(nc.gpsimd, tb[:, 0:Z], vcat[:, Z2:Z3], Es[:, Z2:Z3])
            vmul(nc.vector, tb[:, Z:Z3], vcat[:, 0:Z2], Es[:, 0:Z2])
            p5 = mid.tile([128, 3 * TW], BF16, name="p5")
            nc.vector.tensor_sub(p5[:, :Z3], ta[:, :Z3], tb[:, :Z3])

            p3 = mid.tile([128, 3 * TW], BF16, name="p3")
            vmul(nc.vector, as3(p3[:, :Z3], Z), as3(vcat[:, :Z3], Z),
                 bc3(cs[:, 0:Z], Z))
            t4 = mid.tile([128, 3 * TW], BF16, name="t4")
            vmul(nc.vector, t4[:, :Z3], ds[:, :Z3], vcat[:, :Z3])
            p4 = mid.tile([128, TW], BF16, name="p4")
            nc.gpsimd.tensor_add(p4[:, :Z], t4[:, 0:Z], t4[:, Z:Z2])
            nc.gpsimd.tensor_add(p4[:, :Z], p4[:, :Z], t4[:, Z2:Z3])

            return (z0, Z, p2m, p3, p5, p1, p4)

        hist = []
        for (z0, Z) in zblocks:
            prev = hist.pop(0) if len(hist) >= 2 else None
            hist.append(iteration(z0, Z, prev))
        evs = []
        for si, st in enumerate(hist):
            tg = f"ep{si}"
            evs.append((oev.tile([128, 3 * TW], BF16, name="ev1o",
                                 tag=tg + "a", bufs=1),
                        oev.tile([128, 3 * TW], BF16, name="ev1e",
                                 tag=tg + "b", bufs=1),
                        oev.tile([128, 2 * TW], BF16, name="ev0e",
                                 tag=tg + "c", bufs=1)))
        for j in range(3):
            for st, (e1o, e1e, e0e) in zip(hist, evs):
                out_1o(st, j, e1o)
                out_1e(st, j, e1e)
        for m in range(2):
            for st, (e1o, e1e, e0e) in zip(hist, evs):
                out_0e(st, m, e0e)


def _prep_inputs(node_feat, w_00_0, w_01_1, w_10_1, w_11_0, w_11_1,
                 W_0e, W_1o, W_1e):
    import ml_dtypes
    ndt = ml_dtypes.bfloat16
    weights = {
        "wt000": np.ascontiguousarray((C_000 * w_00_0).T).astype(ndt),
        "wt011": np.ascontiguousarray((C_011 * w_01_1).T).astype(ndt),
        "wt101": np.ascontiguousarray((C_101 * w_10_1).T).astype(ndt),
        "wt110": np.ascontiguousarray((C_110 * w_11_0).T).astype(ndt),
        "wt111": np.ascontiguousarray((C_111 * w_11_1).T).astype(ndt),
        "l0e": np.ascontiguousarray(W_0e / np.sqrt(384.0)).astype(ndt),
        "l1o": np.ascontiguousarray(W_1o / np.sqrt(384.0)).astype(ndt),
        "l1e": np.ascontiguousarray(W_1e / np.sqrt(128.0)).astype(ndt),
        "l1en": np.ascontiguousarray(-W_1e / np.sqrt(128.0)).astype(ndt),
    }
    feat = np.asarray(node_feat, dtype=np.float32).reshape(N_CORES, NS, 640)
    in_maps = []
    for i in range(N_CORES):
        blk = feat[i]
        xT = np.zeros((640, NSH), ndt)
        xT[:256, :NS] = blk[:, :256].T.astype(ndt)
        vv = blk[:, 256:].reshape(NS, 128, 3)
        xT[256:, :NS] = vv.transpose(2, 1, 0).reshape(384, NS).astype(ndt)
        in_maps.append({"xT": xT, **weights})
    return in_maps


def _gather(results):
    out = np.empty((N_NODES, 1024), np.float32)
    for i in range(N_CORES):
        oT = np.asarray(results[i]["outT"]).astype(np.float32,
                                                   copy=False)[:, :NS]
        blk = out[i * NS:(i + 1) * NS]
        blk[:, :256] = oT[:256].T
        blk[:, 256:640] = oT[256:640].reshape(3, 128, NS).transpose(2, 1, 0) \
            .reshape(NS, 384)
        blk[:, 640:] = oT[640:].reshape(3, 128, NS).transpose(2, 1, 0) \
            .reshape(NS, 384)
    return out


def kernel(node_feat, w_00_0, w_01_1, w_10_1, w_11_0, w_11_1,
           W_0e, W_1o, W_1e, _trace=False):
    if "v1" not in _CACHE:
        _CACHE["v1"] = _build_program()
    nc = _CACHE["v1"]
    in_maps = _prep_inputs(node_feat, w_00_0, w_01_1, w_10_1, w_11_0,
                           w_11_1, W_0e, W_1o, W_1e)
    res = run_bass_kernel_spmd(nc, in_maps, core_ids=list(range(N_CORES)),
                               trace=_trace)
    out = _gather(res.results)
    if _trace:
        return out, res
    return out


# revision 30
# speedup vs baseline: 1.6420x; 1.0074x over previous
"""Trainium2 Bass kernel for nn_NodePreTrans (e3nn tensor product + linear).

Data-parallel over nodes: 50000 rows sharded 8 ways (6250/core, padded to
6272).  Channel-major layout; bf16 I/O + bf16 matmuls (f32 PSUM).  Merged
wide elementwise ops on DVE (tensor_tensor 2x_1p for SBUF bf16; PSUM
operands run 1x), merged PSUM evacuations on ACT, FD=512 SBUF muls/adds on
GpSimd.  PSUM: stage-1 in a 3-bank x2 rotating pool, stage-2 in 1-bank x2
tiles (8 banks total).  Two-deep software pipeline: stage-2 of block i-2
is interleaved into stage-1 of block i at PE stall points.
"""

import sys

sys.path.insert(0, "/opt/trn_rl_repo")

import numpy as np

import concourse.bacc as bacc
import concourse.bass as bass
import concourse.mybir as mybir
import concourse.tile as tile
from concourse.bass_utils import run_bass_kernel_spmd

N_NODES = 50000
N_CORES = 8
NS = N_NODES // N_CORES          # 6250 real nodes per core
NSH = 6272                       # padded (12*512 + 128)
TW = 512                         # PSUM bank width in f32

C_000 = 1.0 / np.sqrt(256.0)
C_011 = 1.0 / np.sqrt(128.0)
C_101 = 1.0 / np.sqrt(256.0)
C_110 = 1.0 / np.sqrt(384.0)
C_111 = 1.0 / 16.0

F32 = mybir.dt.float32
BF16 = mybir.dt.bfloat16
AOP = mybir.AluOpType

_CACHE = {}


def _build_program():
    nc = bacc.Bacc("TRN2", target_bir_lowering=False, debug=False,
                   num_devices=N_CORES)

    xT_d = nc.dram_tensor("xT", [640, NSH], BF16, kind="ExternalInput").ap()
    wt000_d = nc.dram_tensor("wt000", [256, 256], BF16, kind="ExternalInput").ap()
    wt011_d = nc.dram_tensor("wt011", [128, 256], BF16, kind="ExternalInput").ap()
    wt101_d = nc.dram_tensor("wt101", [256, 128], BF16, kind="ExternalInput").ap()
    wt110_d = nc.dram_tensor("wt110", [128, 128], BF16, kind="ExternalInput").ap()
    wt111_d = nc.dram_tensor("wt111", [128, 128], BF16, kind="ExternalInput").ap()
    l0e_d = nc.dram_tensor("l0e", [384, 256], BF16, kind="ExternalInput").ap()
    l1o_d = nc.dram_tensor("l1o", [384, 128], BF16, kind="ExternalInput").ap()
    l1e_d = nc.dram_tensor("l1e", [128, 128], BF16, kind="ExternalInput").ap()
    l1en_d = nc.dram_tensor("l1en", [128, 128], BF16, kind="ExternalInput").ap()
    outT_d = nc.dram_tensor("outT", [1024, NSH], BF16, kind="ExternalOutput").ap()

    with tile.TileContext(nc) as tc:
        _emit(tc, nc, xT_d, wt000_d, wt011_d, wt101_d, wt110_d, wt111_d,
              l0e_d, l1o_d, l1e_d, l1en_d, outT_d)

    nc.compile()
    return nc


def _emit(tc, nc, xT_d, wt000_d, wt011_d, wt101_d, wt110_d, wt111_d,
          l0e_d, l1o_d, l1e_d, l1en_d, outT_d):
    zblocks = [(i * 512, 512) for i in range(12)] + [(6144, 128)]

    # plain tensor_tensor: the only 2-tensor op with a 2x_1p uop on HW
    def vmul(eng, out, a, b):
        eng.tensor_mul(out, a, b)

    def as3(ap, Z):
        return ap.rearrange("p (j z) -> p j z", z=Z)

    def bc3(ap, Z):
        # [128, Z] slice -> broadcast [128, 3, Z] (stride 0 over j)
        return ap.rearrange("p (o z) -> p o z", o=1).broadcast_to((128, 3, Z))

    with (
        tc.tile_pool(name="wpool", bufs=1) as wpool,
        tc.tile_pool(name="xin", bufs=4) as xin,
        tc.tile_pool(name="mid", bufs=4) as mid,
        tc.tile_pool(name="oev", bufs=2) as oev,
        tc.tile_pool(name="psX", bufs=2, space="PSUM") as psX,
        tc.tile_pool(name="psY", bufs=2, space="PSUM") as psY,
    ):
        # ---- resident weights (bf16) ------------------------------------
        # issue on GpSimd's queue: ACT's queue must stay free for block 1's
        # Es evac, and Sync's for the x loads
        def wtile(name, dram_ap, rows, cols):
            t = wpool.tile([128, cols], BF16, name=name)
            nc.gpsimd.dma_start(t[:, :], dram_ap[rows:rows + 128, :])
            return t

        w111 = wtile("w111", wt111_d, 0, 128)
        w011 = wtile("w011", wt011_d, 0, 256)
        w000 = [wtile(f"w000_{k}", wt000_d, 128 * k, 256) for k in range(2)]
        w101 = [wtile(f"w101_{k}", wt101_d, 128 * k, 128) for k in range(2)]
        w110 = wtile("w110", wt110_d, 0, 128)
        L1e = wtile("l1e", l1e_d, 0, 128)
        L1en = wtile("l1en", l1en_d, 0, 128)
        L1o = [wtile(f"l1o_{k}", l1o_d, 128 * k, 128) for k in range(3)]
        L0e = [wtile(f"l0e_{k}", l0e_d, 128 * k, 256) for k in range(3)]

        def mm(out, w, rhs, start=True, stop=True):
            nc.tensor.matmul(out, w, rhs, start=start, stop=stop)

        def ps3(name):
            return psX.tile([128, 3 * TW], F32, name=name, tag="x3")

        def ps1(name):
            return psY.tile([128, TW], F32, name=name, tag="y1")

        # ---------------- per-block pieces -------------------------------
        def loads(z0, Z):
            Z2, Z3 = 2 * Z, 3 * Z
            vcat = xin.tile([128, 3 * TW], BF16, name="vcat")
            nc.sync.dma_start(
                as3(vcat[:, :Z3], Z),
                xT_d[256:640, z0:z0 + Z].rearrange("(j p) z -> p j z", p=128))
            scat = xin.tile([128, 2 * TW], BF16, name="scat")
            nc.sync.dma_start(
                as3(scat[:, :Z2], Z),
                xT_d[0:256, z0:z0 + Z].rearrange("(m p) z -> p m z", p=128))
            return vcat, scat

        # stage-2 output groups: 1-bank PSUM tiles, evac on ACT, per-group
        # store on Sync.  Each is a generator-style callable so groups can
        # be interleaved into stage-1's natural PE stall points.
        def store3(ev, z0, Z, row0, ngrp):
            nc.sync.dma_start(
                outT_d[row0:row0 + 128 * ngrp, z0:z0 + Z]
                .rearrange("(j p) z -> p j z", p=128),
                as3(ev[:, :Z * ngrp], Z))

        def out_1o(st, j, ev):
            z0, Z, p2m, p3, p5, p1, p4 = st
            o = ps1(f"o1o_{j}")
            tp1o = [p2m[0], p2m[1], p3]
            for ci in range(3):
                mm(o[:, :Z], L1o[ci][:, :], tp1o[ci][:, j * Z:(j + 1) * Z],
                   start=(ci == 0), stop=(ci == 2))
            nc.scalar.copy(ev[:, j * Z:(j + 1) * Z], o[:, :Z])
            if j == 2:
                store3(ev, z0, Z, 256, 3)

        def out_1e(st, j, ev):
            z0, Z, p2m, p3, p5, p1, p4 = st
            o = ps1(f"o1e_{j}")
            mm(o[:, :Z], L1e[:, :], p5[:, j * Z:(j + 1) * Z])
            nc.scalar.copy(ev[:, j * Z:(j + 1) * Z], o[:, :Z])
            if j == 2:
                store3(ev, z0, Z, 640, 3)

        def out_0e(st, m, ev):
            z0, Z, p2m, p3, p5, p1, p4 = st
            o = ps1(f"o0e_{m}")
            mm(o[:, :Z], L0e[0][:, 128 * m:128 * (m + 1)], p1[:, 0:Z],
               start=True, stop=False)
            mm(o[:, :Z], L0e[1][:, 128 * m:128 * (m + 1)], p1[:, Z:2 * Z],
               start=False, stop=False)
            mm(o[:, :Z], L0e[2][:, 128 * m:128 * (m + 1)], p4[:, :Z],
               start=False, stop=True)
            nc.scalar.copy(ev[:, m * Z:(m + 1) * Z], o[:, :Z])
            if m == 1:
                store3(ev, z0, Z, 0, 2)

        def iteration(z0, Z, prev, io):
            """Emit stage-1 of this block, with stage-2 of the block from
            two iterations ago interleaved into PE wait windows.  The x
            tiles in `io` were DMA-issued one iteration earlier."""
            Z2, Z3 = 2 * Z, 3 * Z
            vcat, scat = io
            if prev is not None:
                ev1o = oev.tile([128, 3 * TW], BF16, name="ev1o", tag="e3")
                ev1e = oev.tile([128, 3 * TW], BF16, name="ev1e", tag="e3")
                ev0e = oev.tile([128, 2 * TW], BF16, name="ev0e", tag="e2")

            # E matmuls; stored [E2|E0|E1] so the p5 cross products are
            # contiguous-range elementwise ops
            Ecat = ps3("Ecat")
            mm(Ecat[:, 0:Z], w111[:, :], vcat[:, Z2:Z3])
            mm(Ecat[:, Z:Z2], w111[:, :], vcat[:, 0:Z])
            mm(Ecat[:, Z2:Z3], w111[:, :], vcat[:, Z:Z2])
            Es = mid.tile([128, 3 * TW], BF16, name="Es")
            nc.scalar.copy(Es[:, :Z3], Ecat[:, :Z3])

            # acM = [a0|a1|c]
            acM = ps3("acM")
            for m in range(2):
                mm(acM[:, m * Z:(m + 1) * Z],
                   w000[0][:, 128 * m:128 * (m + 1)], scat[:, 0:Z],
                   start=True, stop=False)
                mm(acM[:, m * Z:(m + 1) * Z],
                   w000[1][:, 128 * m:128 * (m + 1)], scat[:, Z:Z2],
                   start=False, stop=True)
            mm(acM[:, Z2:Z3], w101[0][:, :], scat[:, 0:Z],
               start=True, stop=False)
            mm(acM[:, Z2:Z3], w101[1][:, :], scat[:, Z:Z2],
               start=False, stop=True)
            p1 = mid.tile([128, 2 * TW], BF16, name="p1")
            vmul(nc.vector, p1[:, :Z2], acM[:, 0:Z2], scat[:, 0:Z2])
            cs = mid.tile([128, TW], BF16, name="cs")
            nc.scalar.copy(cs[:, :Z], acM[:, Z2:Z3])

            if prev is not None:
                out_1o(prev, 0, ev1o)
                out_1e(prev, 0, ev1e)

            # b matmuls for m=0 (p2 mul emitted right after, to free bank)
            bm0 = ps3("bm0")
            for j in range(3):
                mm(bm0[:, j * Z:(j + 1) * Z], w011[:, 0:128],
                   vcat[:, j * Z:(j + 1) * Z])
            p2m = [mid.tile([128, 3 * TW], BF16, name="p2m0")]
            vmul(nc.vector, as3(p2m[0][:, :Z3], Z), as3(bm0[:, :Z3], Z),
                 bc3(scat[:, 0:Z], Z))

            if prev is not None:
                out_1o(prev, 1, ev1o)
                out_1e(prev, 1, ev1e)

            bm1 = ps3("bm1")
            for j in range(3):
                mm(bm1[:, j * Z:(j + 1) * Z], w011[:, 128:256],
                   vcat[:, j * Z:(j + 1) * Z])
            p2m.append(mid.tile([128, 3 * TW], BF16, name="p2m1"))
            vmul(nc.vector, as3(p2m[1][:, :Z3], Z), as3(bm1[:, :Z3], Z),
                 bc3(scat[:, Z:Z2], Z))

            if prev is not None:
                out_1o(prev, 2, ev1o)
                out_1e(prev, 2, ev1e)

            # d matmuls
            dcat = ps3("dcat")
            for j in range(3):
                mm(dcat[:, j * Z:(j + 1) * Z], w110[:, :],
                   vcat[:, j * Z:(j + 1) * Z])
            ds = mid.tile([128, 3 * TW], BF16, name="ds")
            nc.scalar.copy(ds[:, :Z3], dcat[:, :Z3])

            if prev is not None:
                out_0e(prev, 0, ev0e)
                out_0e(prev, 1, ev0e)

            # SBUF-side elementwise (DVE 2x_1p / GpSimd)
            ta = mid.tile([128, 3 * TW], BF16, name="ta")
            vmul(nc.vector, ta[:, 0:Z2], vcat[:, Z:Z3], Es[:, 0:Z2])
            vmul(nc.gpsimd, ta[:, Z2:Z3], vcat[:, 0:Z], Es[:, Z2:Z3])
            tb = mid.tile([128, 3 * TW], BF16, name="tb")
            vmul(nc.gpsimd, tb[:, 0:Z], vcat[:, Z2:Z3], Es[:, Z2:Z3])
            vmul(nc.vector, tb[:, Z:Z3], vcat[:, 0:Z2], Es[:, 0:Z2])
            p5 = mid.tile([128, 3 * TW], BF16, name="p5")
            nc.vector.tensor_sub(p5[:, :Z3], ta[:, :Z3], tb[:, :Z3])

            p3 = mid.tile([128, 3 * TW], BF16, name="p3")
            vmul(nc.vector, as3(p3[:, :Z3], Z), as3(vcat[:, :Z3], Z),
                 bc3(cs[:, 0:Z], Z))
            t4 = mid.tile([128, 3 * TW], BF16, name="t4")
            vmul(nc.vector, t4[:, :Z3], ds[:, :Z3], vcat[:, :Z3])
            p4 = mid.tile([128, TW], BF16, name="p4")
            nc.gpsimd.tensor_add(p4[:, :Z], t4[:, 0:Z], t4[:, Z:Z2])
            nc.gpsimd.tensor_add(p4[:, :Z], p4[:, :Z], t4[:, Z2:Z3])

            return (z0, Z, p2m, p3, p5, p1, p4)

        hist = []
        io = loads(*zblocks[0])
        for bi, (z0, Z) in enumerate(zblocks):
            nxt_io = (loads(*zblocks[bi + 1])
                      if bi + 1 < len(zblocks) else None)
            prev = hist.pop(0) if len(hist) >= 2 else None
            hist.append(iteration(z0, Z, prev, io))
            io = nxt_io
        evs = []
        for si, st in enumerate(hist):
            tg = f"ep{si}"
            evs.append((oev.tile([128, 3 * TW], BF16, name="ev1o",
                                 tag=tg + "a", bufs=1),
                        oev.tile([128, 3 * TW], BF16, name="ev1e",
                                 tag=tg + "b", bufs=1),
                        oev.tile([128, 2 * TW], BF16, name="ev0e",
                                 tag=tg + "c", bufs=1)))
        for j in range(3):
            for st, (e1o, e1e, e0e) in zip(hist, evs):
                out_1o(st, j, e1o)
                out_1e(st, j, e1e)
        for m in range(2):
            for st, (e1o, e1e, e0e) in zip(hist, evs):
                out_0e(st, m, e0e)


def _prep_inputs(node_feat, w_00_0, w_01_1, w_10_1, w_11_0, w_11_1,
                 W_0e, W_1o, W_1e):
    import ml_dtypes
    ndt = ml_dtypes.bfloat16
    weights = {
        "wt000": np.ascontiguousarray((C_000 * w_00_0).T).astype(ndt),
        "wt011": np.ascontiguousarray((C_011 * w_01_1).T).astype(ndt),
        "wt101": np.ascontiguousarray((C_101 * w_10_1).T).astype(ndt),
        "wt110": np.ascontiguousarray((C_110 * w_11_0).T).astype(ndt),
        "wt111": np.ascontiguousarray((C_111 * w_11_1).T).astype(ndt),
        "l0e": np.ascontiguousarray(W_0e / np.sqrt(384.0)).astype(ndt),
        "l1o": np.ascontiguousarray(W_1o / np.sqrt(384.0)).astype(ndt),
        "l1e": np.ascontiguousarray(W_1e / np.sqrt(128.0)).astype(ndt),
        "l1en": np.ascontiguousarray(-W_1e / np.sqrt(128.0)).astype(ndt),
    }
    feat = np.asarray(node_feat, dtype=np.float32).reshape(N_CORES, NS, 640)
    in_maps = []
    for i in range(N_CORES):
        blk = feat[i]
        xT = np.zeros((640, NSH), ndt)
        xT[:256, :NS] = blk[:, :256].T.astype(ndt)
        vv = blk[:, 256:].reshape(NS, 128, 3)
        xT[256:, :NS] = vv.transpose(2, 1, 0).reshape(384, NS).astype(ndt)
        in_maps.append({"xT": xT, **weights})
    return in_maps


def _gather(results):
    out = np.empty((N_NODES, 1024), np.float32)
    for i in range(N_CORES):
        oT = np.asarray(results[i]["outT"]).astype(np.float32,
                                                   copy=False)[:, :NS]
        blk = out[i * NS:(i + 1) * NS]
        blk[:, :256] = oT[:256].T
        blk[:, 256:640] = oT[256:640].reshape(3, 128, NS).transpose(2, 1, 0) \
            .reshape(NS, 384)
        blk[:, 640:] = oT[640:].reshape(3, 128, NS).transpose(2, 1, 0) \
            .reshape(NS, 384)
    return out


def kernel(node_feat, w_00_0, w_01_1, w_10_1, w_11_0, w_11_1,
           W_0e, W_1o, W_1e, _trace=False):
    if "v1" not in _CACHE:
        _CACHE["v1"] = _build_program()
    nc = _CACHE["v1"]
    in_maps = _prep_inputs(node_feat, w_00_0, w_01_1, w_10_1, w_11_0,
                           w_11_1, W_0e, W_1o, W_1e)
    res = run_bass_kernel_spmd(nc, in_maps, core_ids=list(range(N_CORES)),
                               trace=_trace)
    out = _gather(res.results)
    if _trace:
        return out, res
    return out
